# revision 1
# baseline (speedup 1.0000x reference)
"""Trainium2 Bass kernel for 3-layer GNN message passing with per-edge
multi-head attention over node history, distributed over 8 NeuronCores.

Sharding: nodes partitioned across cores by id (2500/core); edges sharded by
TARGET node and col-sorted into 128-edge tiles grouped into 128-target
superblocks. Per layer, per-node projection tables (k/v/q rows) are computed
node-sharded on device, assembled on host between launches, and gathered
per-edge via bulk indirect DMA. Segment-sum is a one-hot matmul accumulating
in PSUM per superblock. 4 launches: proj, layer1, layer2, layer3+head.
"""

import sys
import types

import numpy as np

sys.path.insert(0, "/opt/trn_rl_repo")

# ---------------------------------------------------------------- fixups
_HOOK = [None]


def _install_fixups():
    if "antenv.axon_hooks" not in sys.modules:
        mod = types.ModuleType("antenv.axon_hooks")
        mod.set_axon_ntff_profile_hook = lambda h: _HOOK.__setitem__(0, h)
        mod.get_axon_ntff_profile_hook = lambda: _HOOK[0]
        sys.modules["antenv.axon_hooks"] = mod
        try:
            from trn_agent_boot.trn_boot import _ntff_profile_via_ctypes

            _HOOK[0] = _ntff_profile_via_ctypes("/opt/axon/libaxon_pjrt.so")
        except Exception:
            pass

    import concourse.tile as tile
    from concourse.vector_clock import ScopedClock
    import bass_rust

    if getattr(tile.TileContext, "_drain_split_installed", False):
        return

    def _drain_and_barrier(self, tick_clock, wait_clock):
        nc = self.nc
        drain_inst = nc.sync.drain()
        wait_clock.add_sem_waits(
            drain_inst.ins, ScopedClock({None: tick_clock.global_clock})
        )
        si = drain_inst.ins.sync_info
        waits = list(si.on_wait or []) if si is not None else []
        if len(waits) > 1:
            si.on_wait = waits[:1]
            for i in range(1, len(waits)):
                d2 = nc.sync.drain()
                d2.ins.sync_info = bass_rust.SyncInfo(
                    on_wait=waits[i : i + 1], on_update=[]
                )
        nc.all_engine_barrier()
        assert self.sems is not None
        popped = nc._tile_sem_poison_stack.pop()
        assert popped is self._sem_poison
        nc.clear_and_free_semaphores(list(self.sems.allocated().values()))
        nc.all_engine_barrier()

    tile.TileContext._drain_and_barrier = _drain_and_barrier
    tile.TileContext._drain_split_installed = True


# ---------------------------------------------------------------- constants
N = 20000
E = 320000
IN_C = 256
HID = 64
OUT_C = 64
HEADS = 8
DH = 8
NCORES = 8
NPC = N // NCORES  # 2500
SBT = 128  # targets per superblock
NSB = (NPC + SBT - 1) // SBT  # 20
G = 4  # tiles per compute group
SPAN = 16  # tiles per gather DMA

_CACHE = {}


# ---------------------------------------------------------------- host prep
def _preprocess(edge_index):
    row = np.asarray(edge_index[0], dtype=np.int64)
    col = np.asarray(edge_index[1], dtype=np.int64)
    loop = np.arange(N, dtype=np.int64)
    row_all = np.concatenate([row, loop])
    col_all = np.concatenate([col, loop])
    deg = np.bincount(col_all, minlength=N).astype(np.float32)
    dinv = deg**-0.5
    norm = (dinv[row_all] * dinv[col_all]).astype(np.float32)

    per_core = []
    tps = np.zeros(NSB, dtype=np.int64)
    for c in range(NCORES):
        m = (col_all >= c * NPC) & (col_all < (c + 1) * NPC)
        r = row_all[m]
        co = col_all[m] - c * NPC
        nm = norm[m]
        order = np.argsort(co, kind="stable")
        r, co, nm = r[order], co[order], nm[order]
        counts = np.bincount(co // SBT, minlength=NSB)
        per_core.append((r, co, nm, counts))
        tps = np.maximum(tps, (counts + 127) // 128)
    tps = ((tps + G - 1) // G) * G
    tt = int(tps.sum())
    nspan = (tt + SPAN - 1) // SPAN
    tt_pad = nspan * SPAN

    metas = []
    for c in range(NCORES):
        r, co, nm, counts = per_core[c]
        eidx = np.zeros(tt_pad * 128, dtype=np.int32)
        cidx = np.zeros(tt_pad * 128, dtype=np.int32)
        slot = np.zeros(tt_pad * 128, dtype=np.float32)
        nrm = np.zeros(tt_pad * 128, dtype=np.float32)
        ptr = 0
        tile0 = 0
        for k in range(NSB):
            cnt = int(counts[k])
            base = tile0 * 128
            sl = slice(ptr, ptr + cnt)
            eidx[base : base + cnt] = r[sl]
            cidx[base : base + cnt] = co[sl] + c * NPC
            slot[base : base + cnt] = (co[sl] - k * SBT).astype(np.float32)
            nrm[base : base + cnt] = nm[sl]
            ptr += cnt
            tile0 += int(tps[k])
        metas.append(
            dict(
                eidx=np.ascontiguousarray(eidx.reshape(tt_pad, 128).T),
                cidx=np.ascontiguousarray(cidx.reshape(tt_pad, 128).T),
                slot=np.ascontiguousarray(slot.reshape(tt_pad, 128).T),
                nrm=np.ascontiguousarray(nrm.reshape(tt_pad, 128).T),
            )
        )
    return metas, tps, tt, tt_pad


_WS_CTR = [0]


def _split_multi_waits(nc, maxw=1):
    """This container's walrus rejects instructions with more than one sync
    wait; hoist excess waits onto NoOps inserted before the instruction."""
    from concourse import mybir

    for f in nc.m.functions:
        for bb in f.blocks:
            insts = list(bb.instructions)
            out = []
            changed = False
            for inst in insts:
                si = inst.sync_info
                waits = list(si.on_wait) if (si is not None and si.on_wait) else []
                if len(waits) > maxw:
                    excess = waits[: len(waits) - maxw]
                    for j in range(0, len(excess), maxw):
                        _WS_CTR[0] += 1
                        out.append(
                            mybir.InstNoOp(
                                name=f"waitsplit_{_WS_CTR[0]}",
                                engine=inst.engine,
                                sync_info=mybir.SyncInfo(
                                    on_wait=excess[j : j + maxw], on_update=[]
                                ),
                                bass_nofuse=True,
                            )
                        )
                    si.on_wait = waits[len(waits) - maxw :]
                    changed = True
                out.append(inst)
            if changed:
                bb.instructions = out


# ---------------------------------------------------------------- bass helpers
def _mk_nc():
    import concourse.bass as bass

    return bass.Bass(num_devices=NCORES, debug=False, target_bir_lowering=False)


def _const_tiles(nc, pool):
    from concourse import mybir
    from concourse.masks import make_identity

    iota_i = pool.tile([128, 128], mybir.dt.int32)
    nc.gpsimd.iota(iota_i[:], pattern=[[1, 128]], base=0, channel_multiplier=0)
    iota_f = pool.tile([128, 128], mybir.dt.float32)
    nc.vector.tensor_copy(iota_f[:], iota_i[:])
    ident = pool.tile([128, 128], mybir.dt.float32)
    make_identity(nc, ident[:])
    return iota_f, ident


def _load_w(nc, pool, dram_ap, p, f, tag):
    from concourse import mybir

    t = pool.tile([p, f], mybir.dt.float32, tag=tag)
    nc.sync.dma_start(t[:], dram_ap[:])
    return t


def _proj_to_sbuf(nc, psum_pool, dst, pairs, bias, func, nchunk=500):
    """dst = func(sum_i lhsT_i.T @ rhs_i + bias), streamed over node chunks.

    pairs: list of (lhsT_tile_ap, rhs_fn(j0, w) -> AP).
    """
    from concourse import mybir

    n = dst.shape[1]
    for j0 in range(0, n, nchunk):
        w = min(nchunk, n - j0)
        ps = psum_pool.tile([64, nchunk], mybir.dt.float32, tag="proj")
        for i, (lt, rs) in enumerate(pairs):
            nc.tensor.matmul(
                out=ps[:, :w],
                lhsT=lt,
                rhs=rs(j0, w),
                start=(i == 0),
                stop=(i == len(pairs) - 1),
            )
        nc.scalar.activation(dst[:, j0 : j0 + w], ps[:, :w], func, bias=bias)


def _edge_phase(nc, tc, ctx, meta_aps, table, th, tps, tt_pad, iota_f, out_cb, iden_t=None, pseg_bufs=2):
    """Edge pipeline for history length th (1/2/3).

    Table row layout: [k_0..k_{th-1} | v_0..v_{th-1} | q] * 64 (th>1), or
    [v] (th==1). Calls out_cb(k, psum [128,64]) per superblock (pre-relu).
    """
    from concourse import mybir
    import concourse.bass as bass

    f32 = mybir.dt.float32
    kvw = (2 * th if th > 1 else 1) * 64

    meta_pool = ctx.enter_context(tc.tile_pool(name="meta", bufs=1))
    eidx_t = meta_pool.tile([128, tt_pad], mybir.dt.int32)
    nrm_t = meta_pool.tile([128, tt_pad], f32)
    slot_t = meta_pool.tile([128, tt_pad], f32)
    nc.sync.dma_start(eidx_t[:], meta_aps["eidx"][:])
    nc.sync.dma_start(nrm_t[:], meta_aps["nrm"][:])
    nc.sync.dma_start(slot_t[:], meta_aps["slot"][:])


    gat_pool = ctx.enter_context(tc.tile_pool(name="gat", bufs=2))
    q_pool = ctx.enter_context(tc.tile_pool(name="qg", bufs=2))
    tmp_pool = ctx.enter_context(tc.tile_pool(name="etmp", bufs=2))
    sco_pool = ctx.enter_context(tc.tile_pool(name="esco", bufs=2))
    t_pool = ctx.enter_context(tc.tile_pool(name="tmat", bufs=2))
    psum_out = ctx.enter_context(tc.tile_pool(name="pseg", bufs=pseg_bufs, space="PSUM"))
    if th >= 2:
        tt_psum = ctx.enter_context(tc.tile_pool(name="ttp", bufs=2, space="PSUM"))
        qe_psum = ctx.enter_context(tc.tile_pool(name="qep", bufs=2, space="PSUM"))
        qsb_pool = ctx.enter_context(tc.tile_pool(name="qsb", bufs=2))
        tts_pool = ctx.enter_context(tc.tile_pool(name="tts", bufs=2))

    def issue_span(s):
        # one indirect DMA per 128-edge tile (offset [128,1] is the only
        # pattern the walrus unroll honors); SPAN tiles batched per buffer
        if th == 1:
            kv = gat_pool.tile([128, SPAN, 64], f32, tag="kv")
        else:
            kv = gat_pool.tile([128, SPAN, 2 * th, 8, 8], f32, tag="kv")
        for u in range(SPAN):
            nc.gpsimd.indirect_dma_start(
                out=kv[:, u] if th == 1 else kv[:, u].rearrange("p t a b -> p (t a b)"),
                out_offset=None,
                in_=table[:],
                in_offset=bass.IndirectOffsetOnAxis(
                    ap=eidx_t[:, s * SPAN + u : s * SPAN + u + 1], axis=0
                ),
            )
        return (kv, None)

    spans = {}
    sb_of_tile = []
    for k in range(NSB):
        sb_of_tile += [k] * int(tps[k])
    tt = len(sb_of_tile)

    AT = mybir.AluOpType
    ps = None
    done = 0
    for t0 in range(0, tt, G):
        k = sb_of_tile[t0]
        s = t0 // SPAN
        o = t0 - s * SPAN
        if s not in spans:
            spans[s] = issue_span(s)
        kv, qe = spans[s]
        nrm = nrm_t[:, t0 : t0 + G]

        if th >= 2 and done == 0:
            qsb = qsb_pool.tile([128, HID], f32, tag="qsb")
            j0q = k * SBT
            wq_ = min(SBT, NPC - j0q)
            nc.sync.dma_start(qsb[:wq_], meta_aps["qslice"][j0q : j0q + wq_, :])

        tmat = t_pool.tile([128, G, 128], f32, tag="tmat")
        nc.vector.tensor_tensor(
            out=tmat[:],
            in0=slot_t[:, t0 : t0 + G, None].to_broadcast([128, G, 128]),
            in1=iota_f[:, None, :].to_broadcast([128, G, 128]),
            op=AT.is_equal,
        )

        if th == 1:
            ve3 = kv[:, o : o + G, :]  # [128,G,64]
            msg = tmp_pool.tile([128, G, 64], f32, tag="msg")
            nc.vector.tensor_tensor(
                out=msg[:], in0=ve3,
                in1=nrm[:, :, None].to_broadcast([128, G, 64]), op=AT.mult,
            )
        else:
            ke = kv[:, o : o + G, 0:th]  # [128,G,th,8,8]
            if True:
                ttp = tt_psum.tile([128, G, 128], f32, tag="ttp")
                for gi in range(G):
                    nc.tensor.transpose(
                        out=ttp[:, gi], in_=tmat[:, gi], identity=iden_t[:]
                    )
                tts = tts_pool.tile([128, G, 128], f32, tag="tts")
                nc.scalar.copy(tts[:], ttp[:])
                qep = qe_psum.tile([128, G, 8, 8], f32, tag="qep")
                for gi in range(G):
                    nc.tensor.matmul(
                        out=qep[:, gi], lhsT=tts[:, gi], rhs=qsb[:],
                        start=True, stop=True,
                    )
                q4 = qep[:]
            dmul = tmp_pool.tile([128, G, th, 8, 8], f32, tag="dmul")
            nc.vector.tensor_tensor(
                out=dmul[:], in0=ke,
                in1=qep[:, :, None].to_broadcast([128, G, th, 8, 8]),
                op=AT.mult,
            )
            sc = sco_pool.tile([128, G, th, 8], f32, tag="sc")
            nc.vector.tensor_reduce(
                out=sc[:], in_=dmul[:], axis=mybir.AxisListType.X, op=AT.add
            )
            if th == 2:
                z = sco_pool.tile([128, G, 8], f32, tag="z")
                nc.vector.tensor_tensor(
                    out=z[:], in0=sc[:, :, 0], in1=sc[:, :, 1], op=AT.subtract
                )
                a0 = sco_pool.tile([128, G, 8], f32, tag="a0")
                nc.scalar.activation(
                    a0[:], z[:], mybir.ActivationFunctionType.Sigmoid
                )
                an0 = sco_pool.tile([128, G, 8], f32, tag="an0")
                nc.vector.tensor_tensor(
                    out=an0[:], in0=a0[:],
                    in1=nrm[:, :, None].to_broadcast([128, G, 8]), op=AT.mult,
                )
                an1 = sco_pool.tile([128, G, 8], f32, tag="an1")
                nc.vector.tensor_tensor(
                    out=an1[:],
                    in0=nrm[:, :, None].to_broadcast([128, G, 8]),
                    in1=an0[:], op=AT.subtract,
                )
                aw = [an0, an1]
                wv_ = tmp_pool.tile([128, G, 2, 8, 8], f32, tag="wvp")
                for ti in range(2):
                    nc.vector.tensor_tensor(
                        out=wv_[:, :, ti],
                        in0=kv[:, o : o + G, th + ti],
                        in1=aw[ti][:, :, :, None].to_broadcast([128, G, 8, 8]),
                        op=AT.mult,
                    )
                msg = tmp_pool.tile([128, G, 8, 8], f32, tag="msg2")
                nc.vector.tensor_tensor(
                    out=msg[:], in0=wv_[:, :, 0], in1=wv_[:, :, 1], op=AT.add
                )
            else:
                mx = sco_pool.tile([128, G, 8], f32, tag="mx")
                nc.vector.tensor_tensor(
                    out=mx[:], in0=sc[:, :, 0], in1=sc[:, :, 1], op=AT.max
                )
                nc.vector.tensor_tensor(
                    out=mx[:], in0=mx[:], in1=sc[:, :, 2], op=AT.max
                )
                zz = sco_pool.tile([128, G, th, 8], f32, tag="zz")
                nc.vector.tensor_tensor(
                    out=zz[:], in0=sc[:],
                    in1=mx[:, :, None].to_broadcast([128, G, th, 8]),
                    op=AT.subtract,
                )
                ee = sco_pool.tile([128, G, th, 8], f32, tag="ee")
                nc.scalar.activation(
                    ee[:], zz[:], mybir.ActivationFunctionType.Exp
                )
                dd = sco_pool.tile([128, G, 8], f32, tag="dd")
                nc.vector.tensor_tensor(
                    out=dd[:], in0=ee[:, :, 0], in1=ee[:, :, 1], op=AT.add
                )
                nc.vector.tensor_tensor(
                    out=dd[:], in0=dd[:], in1=ee[:, :, 2], op=AT.add
                )
                rr = sco_pool.tile([128, G, 8], f32, tag="rr")
                nc.vector.reciprocal(rr[:], dd[:])
                rn = sco_pool.tile([128, G, 8], f32, tag="rn")
                nc.vector.tensor_tensor(
                    out=rn[:], in0=rr[:],
                    in1=nrm[:, :, None].to_broadcast([128, G, 8]), op=AT.mult,
                )
                aa = sco_pool.tile([128, G, th, 8], f32, tag="aa")
                nc.vector.tensor_tensor(
                    out=aa[:], in0=ee[:],
                    in1=rn[:, :, None].to_broadcast([128, G, th, 8]), op=AT.mult,
                )
                wv_ = tmp_pool.tile([128, G, th, 8, 8], f32, tag="wvp")
                nc.vector.tensor_tensor(
                    out=wv_[:],
                    in0=kv[:, o : o + G, th : 2 * th],
                    in1=aa[:, :, :, :, None].to_broadcast([128, G, th, 8, 8]),
                    op=AT.mult,
                )
                msg = tmp_pool.tile([128, G, 8, 8], f32, tag="msg2")
                nc.vector.tensor_tensor(
                    out=msg[:], in0=wv_[:, :, 0], in1=wv_[:, :, 1], op=AT.add
                )
                nc.vector.tensor_tensor(
                    out=msg[:], in0=msg[:], in1=wv_[:, :, 2], op=AT.add
                )

        if done == 0:
            ps = psum_out.tile([128, 64], f32, tag="ps")
        for gi in range(G):
            nc.tensor.matmul(
                out=ps[:],
                lhsT=tmat[:, gi],
                rhs=msg[:, gi],
                start=(done + gi == 0),
                stop=(done + gi == int(tps[k]) - 1),
            )
        done += G
        if done == int(tps[k]):
            out_cb(k, ps)
            done = 0


def _meta_dram(nc):
    from concourse import mybir

    f32 = mybir.dt.float32
    i32 = mybir.dt.int32
    return {
        "eidx": nc.dram_tensor("eidx", [128, _TTPAD[0]], i32, kind="ExternalInput").ap(),
        "cidx": nc.dram_tensor("cidx", [128, _TTPAD[0]], i32, kind="ExternalInput").ap(),
        "slot": nc.dram_tensor("slot", [128, _TTPAD[0]], f32, kind="ExternalInput").ap(),
        "nrm": nc.dram_tensor("nrm", [128, _TTPAD[0]], f32, kind="ExternalInput").ap(),
    }


_TTPAD = [None]  # set before building


# ---------------------------------------------------------------- launches
def _build_launch_A():
    import concourse.tile as tile
    from concourse import mybir
    from contextlib import ExitStack

    f32 = mybir.dt.float32
    nc = _mk_nc()
    xT = nc.dram_tensor("xT", [IN_C, NPC], f32, kind="ExternalInput").ap()
    w1 = nc.dram_tensor("w1", [IN_C, HID], f32, kind="ExternalInput").ap()
    b1 = nc.dram_tensor("b1", [HID, 1], f32, kind="ExternalInput").ap()
    wv0 = nc.dram_tensor("wv0", [HID, HID], f32, kind="ExternalInput").ap()
    bv0 = nc.dram_tensor("bv0", [HID, 1], f32, kind="ExternalInput").ap()
    hT_out = nc.dram_tensor("hT_out", [HID, NPC], f32, kind="ExternalOutput").ap()
    v1_rows = nc.dram_tensor("v1_rows", [NPC, HID], f32, kind="ExternalOutput").ap()

    with tile.TileContext(nc) as tc, ExitStack() as ctx:
        cpool = ctx.enter_context(tc.tile_pool(name="const", bufs=1))
        from concourse.masks import make_identity

        ident = cpool.tile([128, 128], f32)
        make_identity(nc, ident[:])

        wpool = ctx.enter_context(tc.tile_pool(name="w", bufs=1))
        xpool = ctx.enter_context(tc.tile_pool(name="x", bufs=2))
        hpool = ctx.enter_context(tc.tile_pool(name="h", bufs=1))
        act_pool = ctx.enter_context(tc.tile_pool(name="act", bufs=2))
        psum_pool = ctx.enter_context(tc.tile_pool(name="ps", bufs=2, space="PSUM"))
        tp_pool = ctx.enter_context(tc.tile_pool(name="tp", bufs=2, space="PSUM"))

        w1a = _load_w(nc, wpool, w1[0:128, :], 128, HID, "w1a")
        w1b = _load_w(nc, wpool, w1[128:256, :], 128, HID, "w1b")
        b1t = _load_w(nc, wpool, b1, HID, 1, "b1t")
        wv0t = _load_w(nc, wpool, wv0, HID, HID, "wv0t")
        bv0t = _load_w(nc, wpool, bv0, HID, 1, "bv0t")

        hT = hpool.tile([HID, NPC], f32)
        v1T = hpool.tile([HID, NPC], f32)

        NCH = 500
        for j0 in range(0, NPC, NCH):
            w = min(NCH, NPC - j0)
            xa = xpool.tile([128, NCH], f32, tag="xa")
            xb = xpool.tile([128, NCH], f32, tag="xb")
            nc.sync.dma_start(xa[:, :w], xT[0:128, j0 : j0 + w])
            nc.sync.dma_start(xb[:, :w], xT[128:256, j0 : j0 + w])
            ps = psum_pool.tile([HID, NCH], f32, tag="p1")
            nc.tensor.matmul(out=ps[:, :w], lhsT=w1a[:], rhs=xa[:, :w], start=True, stop=False)
            nc.tensor.matmul(out=ps[:, :w], lhsT=w1b[:], rhs=xb[:, :w], start=False, stop=True)
            nc.scalar.activation(
                hT[:, j0 : j0 + w], ps[:, :w],
                mybir.ActivationFunctionType.Relu, bias=b1t[:],
            )
            ps2 = psum_pool.tile([HID, NCH], f32, tag="p2")
            nc.tensor.matmul(out=ps2[:, :w], lhsT=wv0t[:], rhs=hT[:, j0 : j0 + w], start=True, stop=True)
            nc.scalar.activation(
                v1T[:, j0 : j0 + w], ps2[:, :w],
                mybir.ActivationFunctionType.Identity, bias=bv0t[:],
            )
        nc.sync.dma_start(hT_out[:], hT[:])

        for j0 in range(0, NPC, 128):
            w = min(128, NPC - j0)
            ps = tp_pool.tile([128, HID], f32, tag="tp")
            nc.tensor.transpose(
                out=ps[:w, :], in_=v1T[:, j0 : j0 + w], identity=ident[:HID, :HID]
            )
            sb = act_pool.tile([128, HID], f32, tag="ro")
            nc.scalar.copy(sb[:w], ps[:w])
            nc.sync.dma_start(v1_rows[j0 : j0 + w, :], sb[:w])
    _split_multi_waits(nc)
    return nc


def _build_launch_mid(layer, tps, tt, tt_pad):
    import concourse.tile as tile
    from concourse import mybir
    from contextlib import ExitStack

    f32 = mybir.dt.float32
    th = layer
    nl = layer + 1
    nc = _mk_nc()
    roww = 64 if th == 1 else (2 * th + 1) * 64
    next_roww = (2 * nl + 1) * 64
    table = nc.dram_tensor("table", [N, roww], f32, kind="ExternalInput").ap()
    meta_aps = _meta_dram(nc)
    if layer == 2:
        meta_aps["qslice"] = nc.dram_tensor(
            "qslice", [NPC, HID], f32, kind="ExternalInput"
        ).ap()
    hists_d = [nc.dram_tensor("histT0", [HID, NPC], f32, kind="ExternalInput").ap()]
    if layer == 2:
        hists_d.append(
            nc.dram_tensor("histT1", [HID, NPC], f32, kind="ExternalInput").ap()
        )
    wk = nc.dram_tensor("wk", [HID, HID], f32, kind="ExternalInput").ap()
    wv = nc.dram_tensor("wv", [HID, HID], f32, kind="ExternalInput").ap()
    wq = nc.dram_tensor("wq", [HID, HID], f32, kind="ExternalInput").ap()
    bk = nc.dram_tensor("bk", [HID, 1], f32, kind="ExternalInput").ap()
    bv = nc.dram_tensor("bv", [HID, 1], f32, kind="ExternalInput").ap()
    bq = nc.dram_tensor("bq", [HID, 1], f32, kind="ExternalInput").ap()
    outT_d = nc.dram_tensor("outT", [HID, NPC], f32, kind="ExternalOutput").ap()
    rows_d = nc.dram_tensor("rows", [NPC, next_roww], f32, kind="ExternalOutput").ap()

    with tile.TileContext(nc) as tc, ExitStack() as ctx:
        cpool = ctx.enter_context(tc.tile_pool(name="const", bufs=1))
        iota_f, ident = _const_tiles(nc, cpool)
        wpool = ctx.enter_context(tc.tile_pool(name="w", bufs=1))
        hpool = ctx.enter_context(tc.tile_pool(name="h", bufs=1))
        act_pool = ctx.enter_context(tc.tile_pool(name="act", bufs=2))
        pb = 1 if layer == 2 else 2
        psum_m = ctx.enter_context(tc.tile_pool(name="psm", bufs=pb, space="PSUM"))
        tp_pool = ctx.enter_context(tc.tile_pool(name="tp", bufs=pb, space="PSUM"))

        wkt = _load_w(nc, wpool, wk, HID, HID, "wkt")
        wvt = _load_w(nc, wpool, wv, HID, HID, "wvt")
        wqt = _load_w(nc, wpool, wq, HID, HID, "wqt")
        bkt = _load_w(nc, wpool, bk, HID, 1, "bkt")
        bvt = _load_w(nc, wpool, bv, HID, 1, "bvt")
        bqt = _load_w(nc, wpool, bq, HID, 1, "bqt")

        histT = []
        for i, hd in enumerate(hists_d):
            ht = hpool.tile([HID, NPC], f32, tag=f"hist{i}")
            nc.sync.dma_start(ht[:], hd[:])
            histT.append(ht)
        outT = hpool.tile([HID, NPC], f32, tag="outT")

        def out_cb(k, ps):
            j0 = k * SBT
            w = min(SBT, NPC - j0)
            sb = act_pool.tile([128, HID], f32, tag="oc")
            nc.scalar.activation(sb[:w], ps[:w], mybir.ActivationFunctionType.Relu)
            tp = tp_pool.tile([HID, 128], f32, tag="ot")
            nc.tensor.transpose(out=tp[:, :w], in_=sb[:w], identity=ident[:w, :w])
            nc.scalar.copy(outT[:, j0 : j0 + w], tp[:, :w])

        _edge_phase(
            nc, tc, ctx, meta_aps, table, th, tps, tt_pad, iota_f, out_cb,
            iden_t=ident, pseg_bufs=(1 if layer == 2 else 2),
        )

        nc.sync.dma_start(outT_d[:], outT[:])

        allh = histT + [outT]
        colTs = []
        Ident = mybir.ActivationFunctionType.Identity
        for i, hsrc in enumerate(allh):
            kt = hpool.tile([HID, NPC], f32, tag=f"kT{i}")
            _proj_to_sbuf(
                nc, psum_m, kt[:],
                [(wkt[:], lambda j0, w, hs=hsrc: hs[:, j0 : j0 + w])], bkt[:], Ident,
            )
            colTs.append(kt)
        for i, hsrc in enumerate(allh):
            vt = hpool.tile([HID, NPC], f32, tag=f"vT{i}")
            _proj_to_sbuf(
                nc, psum_m, vt[:],
                [(wvt[:], lambda j0, w, hs=hsrc: hs[:, j0 : j0 + w])], bvt[:], Ident,
            )
            colTs.append(vt)
        qt = hpool.tile([HID, NPC], f32, tag="qT")
        _proj_to_sbuf(
            nc, psum_m, qt[:],
            [(wqt[:], lambda j0, w: outT[:, j0 : j0 + w])], bqt[:], Ident,
        )
        colTs.append(qt)

        tp2 = ctx.enter_context(tc.tile_pool(name="tp2", bufs=pb, space="PSUM"))
        for j0 in range(0, NPC, 128):
            w = min(128, NPC - j0)
            ps = tp2.tile([128, next_roww], f32, tag="rw")
            for i, ct in enumerate(colTs):
                nc.tensor.transpose(
                    out=ps[:w, i * 64 : (i + 1) * 64],
                    in_=ct[:, j0 : j0 + w],
                    identity=ident[:HID, :HID],
                )
            sb = act_pool.tile([128, next_roww], f32, tag="rwsb")
            nc.scalar.copy(sb[:w], ps[:w])
            nc.sync.dma_start(rows_d[j0 : j0 + w, :], sb[:w])
    _split_multi_waits(nc)
    return nc


def _build_launch_D(tps, tt, tt_pad):
    import concourse.tile as tile
    from concourse import mybir
    from contextlib import ExitStack

    f32 = mybir.dt.float32
    th = 3
    nc = _mk_nc()
    roww = (2 * th + 1) * 64
    table = nc.dram_tensor("table", [N, roww], f32, kind="ExternalInput").ap()
    meta_aps = _meta_dram(nc)
    meta_aps["qslice"] = nc.dram_tensor(
        "qslice", [NPC, HID], f32, kind="ExternalInput"
    ).ap()
    w2 = nc.dram_tensor("w2", [HID, OUT_C], f32, kind="ExternalInput").ap()
    b2bc = nc.dram_tensor("b2bc", [128, OUT_C], f32, kind="ExternalInput").ap()
    y_d = nc.dram_tensor("y", [NPC, OUT_C], f32, kind="ExternalOutput").ap()

    with tile.TileContext(nc) as tc, ExitStack() as ctx:
        cpool = ctx.enter_context(tc.tile_pool(name="const", bufs=1))
        iota_f, ident = _const_tiles(nc, cpool)
        wpool = ctx.enter_context(tc.tile_pool(name="w", bufs=1))
        act_pool = ctx.enter_context(tc.tile_pool(name="act", bufs=2))
        tp_pool = ctx.enter_context(tc.tile_pool(name="tp", bufs=1, space="PSUM"))
        lg_pool = ctx.enter_context(tc.tile_pool(name="lg", bufs=1, space="PSUM"))
        sm_pool = ctx.enter_context(tc.tile_pool(name="sm", bufs=2))

        w2t = _load_w(nc, wpool, w2, HID, OUT_C, "w2t")
        b2t = _load_w(nc, wpool, b2bc, 128, OUT_C, "b2t")
        AT = mybir.AluOpType

        def out_cb(k, ps):
            j0 = k * SBT
            w = min(SBT, NPC - j0)
            o3 = act_pool.tile([128, HID], f32, tag="o3")
            nc.scalar.activation(o3[:w], ps[:w], mybir.ActivationFunctionType.Relu)
            tp = tp_pool.tile([HID, 128], f32, tag="o3t")
            nc.tensor.transpose(out=tp[:, :w], in_=o3[:w], identity=ident[:w, :w])
            o3T = act_pool.tile([HID, 128], f32, tag="o3T")
            nc.scalar.copy(o3T[:, :w], tp[:, :w])
            lg = lg_pool.tile([128, OUT_C], f32, tag="lg")
            nc.tensor.matmul(
                out=lg[:w], lhsT=o3T[:, :w], rhs=w2t[:], start=True, stop=True
            )
            logits = sm_pool.tile([128, OUT_C], f32, tag="logits")
            nc.vector.tensor_tensor(out=logits[:w], in0=lg[:w], in1=b2t[:w], op=AT.add)
            lmax = sm_pool.tile([128, 1], f32, tag="lmax")
            nc.vector.tensor_reduce(
                out=lmax[:w], in_=logits[:w], axis=mybir.AxisListType.X, op=AT.max
            )
            zz = sm_pool.tile([128, OUT_C], f32, tag="zzs")
            nc.vector.tensor_tensor(
                out=zz[:w], in0=logits[:w],
                in1=lmax[:w].to_broadcast([w, OUT_C]), op=AT.subtract,
            )
            eb = sm_pool.tile([128, OUT_C], f32, tag="eb")
            esum = sm_pool.tile([128, 1], f32, tag="esum")
            nc.scalar.activation(
                eb[:w], zz[:w], mybir.ActivationFunctionType.Exp, accum_out=esum[:w]
            )
            lse = sm_pool.tile([128, 1], f32, tag="lse")
            nc.scalar.activation(lse[:w], esum[:w], mybir.ActivationFunctionType.Ln)
            yy = sm_pool.tile([128, OUT_C], f32, tag="yy")
            nc.vector.tensor_tensor(
                out=yy[:w], in0=zz[:w],
                in1=lse[:w].to_broadcast([w, OUT_C]), op=AT.subtract,
            )
            nc.sync.dma_start(y_d[j0 : j0 + w, :], yy[:w])

        _edge_phase(
            nc, tc, ctx, meta_aps, table, th, tps, tt_pad, iota_f, out_cb,
            iden_t=ident,
        )
    _split_multi_waits(nc)
    return nc


# ---------------------------------------------------------------- driver
def kernel(x, edge_index, lin1_w, lin1_b, wq, bq, wk, bk, wv, bv, lin2_w, lin2_b):
    _install_fixups()
    from concourse.bass_utils import run_bass_kernel_spmd

    x = np.asarray(x, dtype=np.float32)
    lin1_w = np.asarray(lin1_w, np.float32)
    lin1_b = np.asarray(lin1_b, np.float32)
    wq = np.asarray(wq, np.float32)
    bq = np.asarray(bq, np.float32)
    wk = np.asarray(wk, np.float32)
    bk = np.asarray(bk, np.float32)
    wv = np.asarray(wv, np.float32)
    bv = np.asarray(bv, np.float32)
    lin2_w = np.asarray(lin2_w, np.float32)
    lin2_b = np.asarray(lin2_b, np.float32)

    metas, tps, tt, tt_pad = _preprocess(np.asarray(edge_index))

    key = ("progs", tuple(tps.tolist()), tt_pad)
    if key not in _CACHE:
        _TTPAD[0] = tt_pad
        _CACHE[key] = (
            _build_launch_A(),
            _build_launch_mid(1, tps, tt, tt_pad),
            _build_launch_mid(2, tps, tt, tt_pad),
            _build_launch_D(tps, tt, tt_pad),
        )
    ncA, ncB, ncC, ncD = _CACHE[key]

    isd = np.float32(1.0 / np.sqrt(DH))
    xT = np.ascontiguousarray(x.T)
    cores = list(range(NCORES))

    in_maps = [
        dict(
            xT=np.ascontiguousarray(xT[:, c * NPC : (c + 1) * NPC]),
            w1=lin1_w,
            b1=lin1_b[:, None],
            wv0=wv[0],
            bv0=bv[0][:, None],
        )
        for c in cores
    ]
    resA = run_bass_kernel_spmd(ncA, in_maps, cores)
    hT = [resA.results[c]["hT_out"] for c in cores]
    v1_table = np.ascontiguousarray(
        np.concatenate([resA.results[c]["v1_rows"] for c in cores], axis=0)
    )

    in_maps = [
        dict(
            table=v1_table,
            eidx=metas[c]["eidx"], cidx=metas[c]["cidx"],
            slot=metas[c]["slot"], nrm=metas[c]["nrm"],
            histT0=hT[c],
            wk=wk[1], wv=wv[1], wq=np.ascontiguousarray(wq[1] * isd),
            bk=bk[1][:, None], bv=bv[1][:, None],
            bq=np.ascontiguousarray((bq[1] * isd))[:, None],
        )
        for c in cores
    ]
    resB = run_bass_kernel_spmd(ncB, in_maps, cores)
    out1T = [resB.results[c]["outT"] for c in cores]
    kvq2_table = np.ascontiguousarray(
        np.concatenate([resB.results[c]["rows"] for c in cores], axis=0)
    )

    in_maps = [
        dict(
            table=kvq2_table,
            eidx=metas[c]["eidx"], cidx=metas[c]["cidx"],
            slot=metas[c]["slot"], nrm=metas[c]["nrm"],
            histT0=hT[c], histT1=out1T[c],
            qslice=np.ascontiguousarray(
                kvq2_table[c * NPC : (c + 1) * NPC, 4 * 64 :]
            ),
            wk=wk[2], wv=wv[2], wq=np.ascontiguousarray(wq[2] * isd),
            bk=bk[2][:, None], bv=bv[2][:, None],
            bq=np.ascontiguousarray((bq[2] * isd))[:, None],
        )
        for c in cores
    ]
    resC = run_bass_kernel_spmd(ncC, in_maps, cores)
    kvq3_table = np.ascontiguousarray(
        np.concatenate([resC.results[c]["rows"] for c in cores], axis=0)
    )

    b2bc = np.ascontiguousarray(np.broadcast_to(lin2_b[None, :], (128, OUT_C)))
    in_maps = [
        dict(
            table=kvq3_table,
            eidx=metas[c]["eidx"], cidx=metas[c]["cidx"],
            slot=metas[c]["slot"], nrm=metas[c]["nrm"],
            qslice=np.ascontiguousarray(
                kvq3_table[c * NPC : (c + 1) * NPC, 6 * 64 :]
            ),
            w2=lin2_w, b2bc=b2bc,
        )
        for c in cores
    ]
    resD = run_bass_kernel_spmd(ncD, in_maps, cores)
    return np.concatenate([resD.results[c]["y"] for c in cores], axis=0)



# revision 7
# speedup vs baseline: 3.2528x; 3.2528x over previous
"""Trainium2 Bass kernel for 3-layer GNN message passing with per-edge
multi-head attention over node history, distributed over 8 NeuronCores.

Sharding: nodes partitioned across cores by id (2500/core); edges sharded by
TARGET node, col-sorted into 128-edge tiles grouped into 128-target
superblocks. Per layer, the per-edge k/v/q rows are assembled on the host
between launches (pure indexing/layout) and streamed to the device as dense
bf16 tiles; all FLOPs (projections, attention, softmax, segment-sum,
activations, head) run on device. Segment-sum is a one-hot matmul (swapped
operands so PSUM holds transposed output columns directly). 4 launches:
proj, layer1, layer2, layer3+head.
"""

import sys
import types

import numpy as np
import ml_dtypes

sys.path.insert(0, "/opt/trn_rl_repo")

BF16 = ml_dtypes.bfloat16

# ---------------------------------------------------------------- fixups
_HOOK = [None]


def _install_fixups():
    if "antenv.axon_hooks" not in sys.modules:
        mod = types.ModuleType("antenv.axon_hooks")
        mod.set_axon_ntff_profile_hook = lambda h: _HOOK.__setitem__(0, h)
        mod.get_axon_ntff_profile_hook = lambda: _HOOK[0]
        sys.modules["antenv.axon_hooks"] = mod
        try:
            from trn_agent_boot.trn_boot import _ntff_profile_via_ctypes

            _HOOK[0] = _ntff_profile_via_ctypes("/opt/axon/libaxon_pjrt.so")
        except Exception:
            pass

    import concourse.tile as tile
    from concourse.vector_clock import ScopedClock
    import bass_rust

    if getattr(tile.TileContext, "_drain_split_installed", False):
        return

    def _drain_and_barrier(self, tick_clock, wait_clock):
        nc = self.nc
        drain_inst = nc.sync.drain()
        wait_clock.add_sem_waits(
            drain_inst.ins, ScopedClock({None: tick_clock.global_clock})
        )
        si = drain_inst.ins.sync_info
        waits = list(si.on_wait or []) if si is not None else []
        if len(waits) > 1:
            si.on_wait = waits[:1]
            for i in range(1, len(waits)):
                d2 = nc.sync.drain()
                d2.ins.sync_info = bass_rust.SyncInfo(
                    on_wait=waits[i : i + 1], on_update=[]
                )
        nc.all_engine_barrier()
        assert self.sems is not None
        popped = nc._tile_sem_poison_stack.pop()
        assert popped is self._sem_poison
        nc.clear_and_free_semaphores(list(self.sems.allocated().values()))
        nc.all_engine_barrier()

    tile.TileContext._drain_and_barrier = _drain_and_barrier
    tile.TileContext._drain_split_installed = True


# ---------------------------------------------------------------- constants
N = 20000
E = 320000
IN_C = 256
HID = 64
OUT_C = 64
HEADS = 8
DH = 8
NCORES = 8
NPC = N // NCORES  # 2500
SBT = 128  # targets per superblock
NSB = (NPC + SBT - 1) // SBT  # 20
CHK = 16  # tiles (of 128 edges) per streamed chunk

_CACHE = {}


# ---------------------------------------------------------------- host prep
def _preprocess(edge_index):
    row = np.asarray(edge_index[0], dtype=np.int64)
    col = np.asarray(edge_index[1], dtype=np.int64)
    loop = np.arange(N, dtype=np.int64)
    row_all = np.concatenate([row, loop])
    col_all = np.concatenate([col, loop])
    deg = np.bincount(col_all, minlength=N).astype(np.float32)
    dinv = np.where(deg > 0, deg**-0.5, 0.0).astype(np.float32)
    norm = (dinv[row_all] * dinv[col_all]).astype(np.float32)
    s_all = np.bincount(col_all, weights=norm.astype(np.float64), minlength=N)
    s_all = s_all.astype(np.float32)

    per_core = []
    tps = np.zeros(NSB, dtype=np.int64)
    for c in range(NCORES):
        m = (col_all >= c * NPC) & (col_all < (c + 1) * NPC)
        r = row_all[m]
        co = col_all[m] - c * NPC
        nm = norm[m]
        order = np.argsort(co, kind="stable")
        r, co, nm = r[order], co[order], nm[order]
        counts = np.bincount(co // SBT, minlength=NSB)
        per_core.append((r, co, nm, counts))
        tps = np.maximum(tps, (counts + 127) // 128)
    tps = np.maximum(tps, 1)
    tt0 = int(tps.sum())
    ttp = ((tt0 + CHK - 1) // CHK) * CHK
    tps[NSB - 1] += ttp - tt0  # padding tiles go to the last superblock

    metas = []
    for c in range(NCORES):
        r, co, nm, counts = per_core[c]
        eidx = np.zeros(ttp * 128, dtype=np.int64)
        cidx = np.zeros(ttp * 128, dtype=np.int64)
        slot = np.zeros(ttp * 128, dtype=np.float32)
        nrm = np.zeros(ttp * 128, dtype=np.float32)
        ptr = 0
        tile0 = 0
        for k in range(NSB):
            cnt = int(counts[k])
            base = tile0 * 128
            sl = slice(ptr, ptr + cnt)
            eidx[base : base + cnt] = r[sl]
            cidx[base : base + cnt] = co[sl]
            slot[base : base + cnt] = (co[sl] - k * SBT).astype(np.float32)
            nrm[base : base + cnt] = nm[sl]
            ptr += cnt
            tile0 += int(tps[k])
        nrm2 = np.ascontiguousarray(nrm.reshape(ttp, 128).T)
        metas.append(
            dict(
                eidx=np.ascontiguousarray(eidx.reshape(ttp, 128).T),
                cidx=np.ascontiguousarray(cidx.reshape(ttp, 128).T),
                slot=np.ascontiguousarray(slot.reshape(ttp, 128).T),
                nrm=nrm2,
                nrmb=nrm2.astype(BF16),
            )
        )
    return metas, tps, ttp, s_all


_WS_CTR = [0]


def _split_multi_waits(nc, maxw=1):
    """This container's walrus rejects instructions with more than one sync
    wait; hoist excess waits onto NoOps inserted before the instruction."""
    from concourse import mybir

    for f in nc.m.functions:
        for bb in f.blocks:
            insts = list(bb.instructions)
            out = []
            changed = False
            for inst in insts:
                si = inst.sync_info
                waits = list(si.on_wait) if (si is not None and si.on_wait) else []
                if len(waits) > maxw:
                    excess = waits[: len(waits) - maxw]
                    for j in range(0, len(excess), maxw):
                        _WS_CTR[0] += 1
                        out.append(
                            mybir.InstNoOp(
                                name=f"waitsplit_{_WS_CTR[0]}",
                                engine=inst.engine,
                                sync_info=mybir.SyncInfo(
                                    on_wait=excess[j : j + maxw], on_update=[]
                                ),
                                bass_nofuse=True,
                            )
                        )
                    si.on_wait = waits[len(waits) - maxw :]
                    changed = True
                out.append(inst)
            if changed:
                bb.instructions = out


# ---------------------------------------------------------------- bass helpers
def _mk_nc():
    import concourse.bass as bass

    return bass.Bass(num_devices=NCORES, debug=False, target_bir_lowering=False)


def _load_w(nc, pool, dram_ap, p, f, tag, dtype=None):
    from concourse import mybir

    t = pool.tile([p, f], dtype or mybir.dt.float32, tag=tag)
    nc.sync.dma_start(t[:], dram_ap[:])
    return t


def _sb_maps(tps):
    sb_of_tile = []
    for k in range(NSB):
        sb_of_tile += [k] * int(tps[k])
    first = {}
    last = {}
    for t, k in enumerate(sb_of_tile):
        if k not in first:
            first[k] = t
        last[k] = t
    return sb_of_tile, first, last


def _proj_cols(nc, tc, ctx, w_t, b_t, srcs, out_slices, act_pool, psum_pool, func=None):
    """For each (src columnar tile [64, NPC], dram slice) pair: write
    func(w.T @ src + b) in bf16 to the dram slice, chunked by 500 cols."""
    from concourse import mybir

    f32 = mybir.dt.float32
    bf = mybir.dt.bfloat16
    Ident = mybir.ActivationFunctionType.Identity
    NCH = 500
    for (src, dst) in zip(srcs, out_slices):
        for j0 in range(0, NPC, NCH):
            w = min(NCH, NPC - j0)
            ps = psum_pool.tile([HID, NCH], f32, tag="proj")
            nc.tensor.matmul(
                out=ps[:, :w], lhsT=w_t[:], rhs=src[:, j0 : j0 + w],
                start=True, stop=True,
            )
            sb = act_pool.tile([HID, NCH], bf, tag="projsb")
            nc.scalar.activation(sb[:, :w], ps[:, :w], func or Ident, bias=b_t[:])
            nc.sync.dma_start(dst[:, j0 : j0 + w], sb[:, :w])


# ---------------------------------------------------------------- launch A
def _build_launch_A():
    import concourse.tile as tile
    from concourse import mybir
    from contextlib import ExitStack

    f32 = mybir.dt.float32
    bf = mybir.dt.bfloat16
    nc = _mk_nc()
    xT = nc.dram_tensor("xT", [IN_C, NPC], bf, kind="ExternalInput").ap()
    w1 = nc.dram_tensor("w1", [IN_C, HID], bf, kind="ExternalInput").ap()
    b1 = nc.dram_tensor("b1", [HID, 1], f32, kind="ExternalInput").ap()
    hT_out = nc.dram_tensor("hT_out", [HID, NPC], bf, kind="ExternalOutput").ap()

    with tile.TileContext(nc) as tc, ExitStack() as ctx:
        wpool = ctx.enter_context(tc.tile_pool(name="w", bufs=1))
        xpool = ctx.enter_context(tc.tile_pool(name="x", bufs=1))
        hpool = ctx.enter_context(tc.tile_pool(name="h", bufs=1))
        psum_pool = ctx.enter_context(tc.tile_pool(name="ps", bufs=2, space="PSUM"))

        w1a = _load_w(nc, wpool, w1[0:128, :], 128, HID, "w1a", bf)
        w1b = _load_w(nc, wpool, w1[128:256, :], 128, HID, "w1b", bf)
        b1t = _load_w(nc, wpool, b1, HID, 1, "b1t")
        xa = xpool.tile([128, NPC], bf, tag="xa")
        xb = xpool.tile([128, NPC], bf, tag="xb")
        nc.sync.dma_start(xa[:], xT[0:128, :])
        nc.sync.dma_start(xb[:], xT[128:256, :])

        hT = hpool.tile([HID, NPC], bf)
        NCH = 500
        Relu = mybir.ActivationFunctionType.Relu
        for j0 in range(0, NPC, NCH):
            w = min(NCH, NPC - j0)
            ps = psum_pool.tile([HID, NCH], f32, tag="p1")
            nc.tensor.matmul(out=ps[:, :w], lhsT=w1a[:], rhs=xa[:, j0 : j0 + w], start=True, stop=False)
            nc.tensor.matmul(out=ps[:, :w], lhsT=w1b[:], rhs=xb[:, j0 : j0 + w], start=False, stop=True)
            nc.scalar.activation(hT[:, j0 : j0 + w], ps[:, :w], Relu, bias=b1t[:])
        nc.sync.dma_start(hT_out[:], hT[:])
    _split_multi_waits(nc)
    return nc


# ---------------------------------------------------------------- edge phase
def _edge_loop(nc, tc, ctx, tps, ttp, ed_ap, slot_t, iota_f, compute_msg, out_cb):
    """Stream edge chunks; compute_msg(ed_t, t0) -> msg tile [128, CHK, 64]
    (bf16); segment-sum via swapped one-hot matmul into psT [64, 128]."""
    from concourse import mybir

    f32 = mybir.dt.float32
    bf = mybir.dt.bfloat16
    AT = mybir.AluOpType
    sb_of_tile, sb_first, sb_last = _sb_maps(tps)

    roww = ed_ap.shape[2]
    ed_pool = ctx.enter_context(tc.tile_pool(name="ed", bufs=2))
    tm_pool = ctx.enter_context(tc.tile_pool(name="tm", bufs=2))
    psum_seg = ctx.enter_context(tc.tile_pool(name="pseg", bufs=2, space="PSUM"))

    psT = None
    for t0 in range(0, ttp, CHK):
        ed_t = ed_pool.tile([128, CHK, roww], bf, tag="ed")
        nc.sync.dma_start(ed_t[:], ed_ap[:, t0 : t0 + CHK, :])

        tm = tm_pool.tile([128, CHK, 128], bf, tag="tm")
        nc.vector.tensor_tensor(
            out=tm[:],
            in0=slot_t[:, t0 : t0 + CHK, None].to_broadcast([128, CHK, 128]),
            in1=iota_f[:, None, :].to_broadcast([128, CHK, 128]),
            op=AT.is_equal,
        )

        msg = compute_msg(ed_t, t0)

        for gi in range(CHK):
            k = sb_of_tile[t0 + gi]
            if t0 + gi == sb_first[k]:
                psT = psum_seg.tile([HID, 128], f32, tag="psT")
            nc.tensor.matmul(
                out=psT[:],
                lhsT=msg[:, gi],
                rhs=tm[:, gi],
                start=(t0 + gi == sb_first[k]),
                stop=(t0 + gi == sb_last[k]),
            )
            if t0 + gi == sb_last[k]:
                out_cb(k, psT)


def _meta_tiles(nc, tc, ctx, ttp, slot_d, nrm_d=None, nrmb_d=None):
    from concourse import mybir

    f32 = mybir.dt.float32
    bf = mybir.dt.bfloat16
    meta_pool = ctx.enter_context(tc.tile_pool(name="meta", bufs=1))
    slot_t = meta_pool.tile([128, ttp], f32)
    nc.sync.dma_start(slot_t[:], slot_d[:])
    nrm_t = None
    if nrm_d is not None:
        nrm_t = meta_pool.tile([128, ttp], f32, tag="nrmf")
        nc.sync.dma_start(nrm_t[:], nrm_d[:])
    nrmb_t = None
    if nrmb_d is not None:
        nrmb_t = meta_pool.tile([128, ttp], bf, tag="nrmb")
        nc.sync.dma_start(nrmb_t[:], nrmb_d[:])
    iota_i = meta_pool.tile([128, 128], mybir.dt.int32, tag="iotai")
    nc.gpsimd.iota(iota_i[:], pattern=[[1, 128]], base=0, channel_multiplier=0)
    iota_f = meta_pool.tile([128, 128], f32, tag="iotaf")
    nc.vector.tensor_copy(iota_f[:], iota_i[:])
    return slot_t, nrm_t, nrmb_t, iota_f


# ---------------------------------------------------------------- launch B (layer 1)
def _build_launch_B(tps, ttp):
    import concourse.tile as tile
    from concourse import mybir
    from contextlib import ExitStack

    f32 = mybir.dt.float32
    bf = mybir.dt.bfloat16
    AT = mybir.AluOpType
    Relu = mybir.ActivationFunctionType.Relu
    nc = _mk_nc()

    ed_d = nc.dram_tensor("ed", [128, ttp, HID], bf, kind="ExternalInput").ap()
    slot_d = nc.dram_tensor("slot", [128, ttp], f32, kind="ExternalInput").ap()
    nrmb_d = nc.dram_tensor("nrmb", [128, ttp], bf, kind="ExternalInput").ap()
    hT_d = nc.dram_tensor("hT", [HID, NPC], bf, kind="ExternalInput").ap()
    s8_d = nc.dram_tensor("s8", [8, NPC], bf, kind="ExternalInput").ap()
    bv08_d = nc.dram_tensor("bv08", [8, HID], bf, kind="ExternalInput").ap()
    wv0_d = nc.dram_tensor("wv0", [HID, HID], bf, kind="ExternalInput").ap()
    wk2_d = nc.dram_tensor("wk2", [HID, HID], bf, kind="ExternalInput").ap()
    wv2_d = nc.dram_tensor("wv2", [HID, HID], bf, kind="ExternalInput").ap()
    wq2_d = nc.dram_tensor("wq2", [HID, HID], bf, kind="ExternalInput").ap()
    bk2_d = nc.dram_tensor("bk2", [HID, 1], f32, kind="ExternalInput").ap()
    bv2_d = nc.dram_tensor("bv2", [HID, 1], f32, kind="ExternalInput").ap()
    bq2_d = nc.dram_tensor("bq2", [HID, 1], f32, kind="ExternalInput").ap()
    outT_d = nc.dram_tensor("outT", [HID, NPC], bf, kind="ExternalOutput").ap()
    cols_d = nc.dram_tensor("cols", [5 * HID, NPC], bf, kind="ExternalOutput").ap()

    with tile.TileContext(nc) as tc, ExitStack() as ctx:
        slot_t, _, nrmb_t, iota_f = _meta_tiles(nc, tc, ctx, ttp, slot_d, nrmb_d=nrmb_d)
        wpool = ctx.enter_context(tc.tile_pool(name="w", bufs=1))
        hpool = ctx.enter_context(tc.tile_pool(name="h", bufs=1))
        msg_pool = ctx.enter_context(tc.tile_pool(name="msg", bufs=2))
        act_pool = ctx.enter_context(tc.tile_pool(name="act", bufs=2))
        psum_o = ctx.enter_context(tc.tile_pool(name="po", bufs=2, space="PSUM"))
        psum_m = ctx.enter_context(tc.tile_pool(name="pm", bufs=2, space="PSUM"))

        wv0t = _load_w(nc, wpool, wv0_d, HID, HID, "wv0t", bf)
        wk2t = _load_w(nc, wpool, wk2_d, HID, HID, "wk2t", bf)
        wv2t = _load_w(nc, wpool, wv2_d, HID, HID, "wv2t", bf)
        wq2t = _load_w(nc, wpool, wq2_d, HID, HID, "wq2t", bf)
        bk2t = _load_w(nc, wpool, bk2_d, HID, 1, "bk2t")
        bv2t = _load_w(nc, wpool, bv2_d, HID, 1, "bv2t")
        bq2t = _load_w(nc, wpool, bq2_d, HID, 1, "bq2t")
        bv08t = _load_w(nc, wpool, bv08_d, 8, HID, "bv08t", bf)
        s8t = _load_w(nc, wpool, s8_d, 8, NPC, "s8t", bf)
        hT = hpool.tile([HID, NPC], bf, tag="hT")
        nc.sync.dma_start(hT[:], hT_d[:])
        outT = hpool.tile([HID, NPC], bf, tag="outT")

        def compute_msg(ed_t, t0):
            msg = msg_pool.tile([128, CHK, HID], bf, tag="msg")
            nc.vector.tensor_tensor(
                out=msg[:],
                in0=ed_t[:],
                in1=nrmb_t[:, t0 : t0 + CHK, None].to_broadcast([128, CHK, HID]),
                op=AT.mult,
            )
            return msg

        def out_cb(k, psT):
            j0 = k * SBT
            w = min(SBT, NPC - j0)
            ST = act_pool.tile([HID, 128], bf, tag="ST")
            nc.scalar.copy(ST[:, :w], psT[:, :w])
            ps2 = psum_o.tile([HID, 128], f32, tag="ps2")
            nc.tensor.matmul(out=ps2[:, :w], lhsT=wv0t[:], rhs=ST[:, :w], start=True, stop=False)
            nc.tensor.matmul(out=ps2[:, :w], lhsT=bv08t[:], rhs=s8t[:, j0 : j0 + w], start=False, stop=True)
            nc.scalar.activation(outT[:, j0 : j0 + w], ps2[:, :w], Relu)

        _edge_loop(nc, tc, ctx, tps, ttp, ed_d, slot_t, iota_f, compute_msg, out_cb)

        _proj_cols(
            nc, tc, ctx, wk2t, bk2t, [hT, outT],
            [cols_d[0:64, :], cols_d[64:128, :]], act_pool, psum_m,
        )
        _proj_cols(
            nc, tc, ctx, wv2t, bv2t, [hT, outT],
            [cols_d[128:192, :], cols_d[192:256, :]], act_pool, psum_m,
        )
        _proj_cols(
            nc, tc, ctx, wq2t, bq2t, [outT], [cols_d[256:320, :]], act_pool, psum_m,
        )
        nc.sync.dma_start(outT_d[:], outT[:])
    _split_multi_waits(nc)
    return nc


# ---------------------------------------------------------------- launch C (layer 2)
def _build_launch_C(tps, ttp):
    import concourse.tile as tile
    from concourse import mybir
    from contextlib import ExitStack

    f32 = mybir.dt.float32
    bf = mybir.dt.bfloat16
    AT = mybir.AluOpType
    Relu = mybir.ActivationFunctionType.Relu
    Sig = mybir.ActivationFunctionType.Sigmoid
    nc = _mk_nc()
    th = 2
    roww = (2 * th + 1) * HID  # 320

    ed_d = nc.dram_tensor("ed", [128, ttp, roww], bf, kind="ExternalInput").ap()
    slot_d = nc.dram_tensor("slot", [128, ttp], f32, kind="ExternalInput").ap()
    nrm_d = nc.dram_tensor("nrm", [128, ttp], f32, kind="ExternalInput").ap()
    hT_d = nc.dram_tensor("hT", [HID, NPC], bf, kind="ExternalInput").ap()
    o1T_d = nc.dram_tensor("o1T", [HID, NPC], bf, kind="ExternalInput").ap()
    wk3_d = nc.dram_tensor("wk3", [HID, HID], bf, kind="ExternalInput").ap()
    wv3_d = nc.dram_tensor("wv3", [HID, HID], bf, kind="ExternalInput").ap()
    wq3_d = nc.dram_tensor("wq3", [HID, HID], bf, kind="ExternalInput").ap()
    bk3_d = nc.dram_tensor("bk3", [HID, 1], f32, kind="ExternalInput").ap()
    bv3_d = nc.dram_tensor("bv3", [HID, 1], f32, kind="ExternalInput").ap()
    bq3_d = nc.dram_tensor("bq3", [HID, 1], f32, kind="ExternalInput").ap()
    cols_d = nc.dram_tensor("cols", [7 * HID, NPC], bf, kind="ExternalOutput").ap()

    with tile.TileContext(nc) as tc, ExitStack() as ctx:
        slot_t, nrm_t, _, iota_f = _meta_tiles(nc, tc, ctx, ttp, slot_d, nrm_d=nrm_d)
        wpool = ctx.enter_context(tc.tile_pool(name="w", bufs=1))
        hpool = ctx.enter_context(tc.tile_pool(name="h", bufs=1))
        dk_pool = ctx.enter_context(tc.tile_pool(name="dk", bufs=2))
        sc_pool = ctx.enter_context(tc.tile_pool(name="sc", bufs=2))
        msg_pool = ctx.enter_context(tc.tile_pool(name="msg", bufs=2))
        act_pool = ctx.enter_context(tc.tile_pool(name="act", bufs=2))
        psum_m = ctx.enter_context(tc.tile_pool(name="pm", bufs=2, space="PSUM"))

        wk3t = _load_w(nc, wpool, wk3_d, HID, HID, "wk3t", bf)
        wv3t = _load_w(nc, wpool, wv3_d, HID, HID, "wv3t", bf)
        wq3t = _load_w(nc, wpool, wq3_d, HID, HID, "wq3t", bf)
        bk3t = _load_w(nc, wpool, bk3_d, HID, 1, "bk3t")
        bv3t = _load_w(nc, wpool, bv3_d, HID, 1, "bv3t")
        bq3t = _load_w(nc, wpool, bq3_d, HID, 1, "bq3t")
        hT = hpool.tile([HID, NPC], bf, tag="hT")
        nc.sync.dma_start(hT[:], hT_d[:])
        o1T = hpool.tile([HID, NPC], bf, tag="o1T")
        nc.sync.dma_start(o1T[:], o1T_d[:])
        o2T = hpool.tile([HID, NPC], bf, tag="o2T")

        def compute_msg(ed_t, t0):
            # ed_t [128, CHK, 320] = [k0 k1 v0 v1 q] * 64
            ke = ed_t[:, :, 0 : 2 * HID].rearrange("p c (t d) -> p c t d", t=2)
            qe = ed_t[:, :, 4 * HID : 5 * HID]
            dk = dk_pool.tile([128, CHK, 2, HID], bf, tag="dk")
            nc.vector.tensor_tensor(
                out=dk[:],
                in0=ke,
                in1=qe[:, :, None, :].to_broadcast([128, CHK, 2, HID]),
                op=AT.mult,
            )
            sc = sc_pool.tile([128, CHK, 2, 8], f32, tag="sc")
            nc.vector.tensor_reduce(
                out=sc[:],
                in_=dk[:].rearrange("p c t (h d) -> p c t h d", h=8),
                axis=mybir.AxisListType.X,
                op=AT.add,
            )
            z = sc_pool.tile([128, CHK, 8], f32, tag="z")
            nc.vector.tensor_tensor(out=z[:], in0=sc[:, :, 0], in1=sc[:, :, 1], op=AT.subtract)
            a0 = sc_pool.tile([128, CHK, 8], f32, tag="a0")
            nc.scalar.activation(a0[:], z[:], Sig)
            an0 = sc_pool.tile([128, CHK, 8], bf, tag="an0")
            nc.vector.tensor_tensor(
                out=an0[:], in0=a0[:],
                in1=nrm_t[:, t0 : t0 + CHK, None].to_broadcast([128, CHK, 8]),
                op=AT.mult,
            )
            a1 = sc_pool.tile([128, CHK, 8], f32, tag="a1")
            nc.scalar.activation(a1[:], z[:], Sig, scale=-1.0)
            an1 = sc_pool.tile([128, CHK, 8], bf, tag="an1")
            nc.vector.tensor_tensor(
                out=an1[:], in0=a1[:],
                in1=nrm_t[:, t0 : t0 + CHK, None].to_broadcast([128, CHK, 8]),
                op=AT.mult,
            )
            ve = ed_t[:, :, 2 * HID : 4 * HID].rearrange(
                "p c (t h d) -> p c t h d", t=2, h=8
            )
            wv0_ = msg_pool.tile([128, CHK, 8, 8], bf, tag="wv0")
            nc.vector.tensor_tensor(
                out=wv0_[:], in0=ve[:, :, 0],
                in1=an0[:, :, :, None].to_broadcast([128, CHK, 8, 8]),
                op=AT.mult,
            )
            wv1_ = msg_pool.tile([128, CHK, 8, 8], bf, tag="wv1")
            nc.vector.tensor_tensor(
                out=wv1_[:], in0=ve[:, :, 1],
                in1=an1[:, :, :, None].to_broadcast([128, CHK, 8, 8]),
                op=AT.mult,
            )
            msg = msg_pool.tile([128, CHK, HID], bf, tag="msg")
            nc.vector.tensor_tensor(
                out=msg[:].rearrange("p c (h d) -> p c h d", h=8),
                in0=wv0_[:], in1=wv1_[:], op=AT.add,
            )
            return msg

        def out_cb(k, psT):
            j0 = k * SBT
            w = min(SBT, NPC - j0)
            nc.scalar.activation(o2T[:, j0 : j0 + w], psT[:, :w], Relu)

        _edge_loop(nc, tc, ctx, tps, ttp, ed_d, slot_t, iota_f, compute_msg, out_cb)

        _proj_cols(
            nc, tc, ctx, wk3t, bk3t, [hT, o1T, o2T],
            [cols_d[0:64, :], cols_d[64:128, :], cols_d[128:192, :]],
            act_pool, psum_m,
        )
        _proj_cols(
            nc, tc, ctx, wv3t, bv3t, [hT, o1T, o2T],
            [cols_d[192:256, :], cols_d[256:320, :], cols_d[320:384, :]],
            act_pool, psum_m,
        )
        _proj_cols(
            nc, tc, ctx, wq3t, bq3t, [o2T], [cols_d[384:448, :]], act_pool, psum_m,
        )
    _split_multi_waits(nc)
    return nc


# ---------------------------------------------------------------- launch D (layer 3 + head)
def _build_launch_D(tps, ttp):
    import concourse.tile as tile
    from concourse import mybir
    from contextlib import ExitStack

    f32 = mybir.dt.float32
    bf = mybir.dt.bfloat16
    AT = mybir.AluOpType
    Relu = mybir.ActivationFunctionType.Relu
    Exp = mybir.ActivationFunctionType.Exp
    Ln = mybir.ActivationFunctionType.Ln
    nc = _mk_nc()
    th = 3
    roww = (2 * th + 1) * HID  # 448

    ed_d = nc.dram_tensor("ed", [128, ttp, roww], bf, kind="ExternalInput").ap()
    slot_d = nc.dram_tensor("slot", [128, ttp], f32, kind="ExternalInput").ap()
    nrm_d = nc.dram_tensor("nrm", [128, ttp], f32, kind="ExternalInput").ap()
    w2_d = nc.dram_tensor("w2", [HID, OUT_C], bf, kind="ExternalInput").ap()
    b2bc_d = nc.dram_tensor("b2bc", [128, OUT_C], f32, kind="ExternalInput").ap()
    y_d = nc.dram_tensor("y", [NPC, OUT_C], f32, kind="ExternalOutput").ap()

    with tile.TileContext(nc) as tc, ExitStack() as ctx:
        slot_t, nrm_t, _, iota_f = _meta_tiles(nc, tc, ctx, ttp, slot_d, nrm_d=nrm_d)
        wpool = ctx.enter_context(tc.tile_pool(name="w", bufs=1))
        dk_pool = ctx.enter_context(tc.tile_pool(name="dk", bufs=2))
        sc_pool = ctx.enter_context(tc.tile_pool(name="sc", bufs=2))
        msg_pool = ctx.enter_context(tc.tile_pool(name="msg", bufs=2))
        act_pool = ctx.enter_context(tc.tile_pool(name="act", bufs=2))
        sm_pool = ctx.enter_context(tc.tile_pool(name="sm", bufs=2))
        psum_lg = ctx.enter_context(tc.tile_pool(name="plg", bufs=2, space="PSUM"))

        w2t = _load_w(nc, wpool, w2_d, HID, OUT_C, "w2t", bf)
        b2t = _load_w(nc, wpool, b2bc_d, 128, OUT_C, "b2t")

        def compute_msg(ed_t, t0):
            ke = ed_t[:, :, 0 : 3 * HID].rearrange("p c (t d) -> p c t d", t=3)
            qe = ed_t[:, :, 6 * HID : 7 * HID]
            dk = dk_pool.tile([128, CHK, 3, HID], bf, tag="dk")
            nc.vector.tensor_tensor(
                out=dk[:],
                in0=ke,
                in1=qe[:, :, None, :].to_broadcast([128, CHK, 3, HID]),
                op=AT.mult,
            )
            sc = sc_pool.tile([128, CHK, 3, 8], f32, tag="sc")
            nc.vector.tensor_reduce(
                out=sc[:],
                in_=dk[:].rearrange("p c t (h d) -> p c t h d", h=8),
                axis=mybir.AxisListType.X,
                op=AT.add,
            )
            ee = sc_pool.tile([128, CHK, 3, 8], bf, tag="ee")
            nc.scalar.activation(ee[:], sc[:], Exp)
            dd1 = sc_pool.tile([128, CHK, 8], bf, tag="dd1")
            nc.vector.tensor_tensor(out=dd1[:], in0=ee[:, :, 0], in1=ee[:, :, 1], op=AT.add)
            dd = sc_pool.tile([128, CHK, 8], f32, tag="dd")
            nc.vector.tensor_tensor(out=dd[:], in0=dd1[:], in1=ee[:, :, 2], op=AT.add)
            rr = sc_pool.tile([128, CHK, 8], f32, tag="rr")
            nc.vector.reciprocal(rr[:], dd[:])
            rn = sc_pool.tile([128, CHK, 8], bf, tag="rn")
            nc.vector.tensor_tensor(
                out=rn[:], in0=rr[:],
                in1=nrm_t[:, t0 : t0 + CHK, None].to_broadcast([128, CHK, 8]),
                op=AT.mult,
            )
            aa = sc_pool.tile([128, CHK, 3, 8], bf, tag="aa")
            nc.vector.tensor_tensor(
                out=aa[:], in0=ee[:],
                in1=rn[:, :, None, :].to_broadcast([128, CHK, 3, 8]),
                op=AT.mult,
            )
            ve = ed_t[:, :, 3 * HID : 6 * HID].rearrange(
                "p c (t h d) -> p c t h d", t=3, h=8
            )
            wv_ = msg_pool.tile([128, CHK, 3, 8, 8], bf, tag="wv")
            nc.vector.tensor_tensor(
                out=wv_[:], in0=ve,
                in1=aa[:, :, :, :, None].to_broadcast([128, CHK, 3, 8, 8]),
                op=AT.mult,
            )
            msg1 = msg_pool.tile([128, CHK, 8, 8], bf, tag="msg1")
            nc.vector.tensor_tensor(out=msg1[:], in0=wv_[:, :, 0], in1=wv_[:, :, 1], op=AT.add)
            msg = msg_pool.tile([128, CHK, HID], bf, tag="msg")
            nc.vector.tensor_tensor(
                out=msg[:].rearrange("p c (h d) -> p c h d", h=8),
                in0=msg1[:], in1=wv_[:, :, 2], op=AT.add,
            )
            return msg

        def out_cb(k, psT):
            j0 = k * SBT
            w = min(SBT, NPC - j0)
            o3T = act_pool.tile([HID, 128], bf, tag="o3T")
            nc.scalar.activation(o3T[:, :w], psT[:, :w], Relu)
            lg = psum_lg.tile([128, OUT_C], f32, tag="lg")
            nc.tensor.matmul(out=lg[:w], lhsT=o3T[:, :w], rhs=w2t[:], start=True, stop=True)
            logits = sm_pool.tile([128, OUT_C], f32, tag="logits")
            nc.vector.tensor_tensor(out=logits[:w], in0=lg[:w], in1=b2t[:w], op=AT.add)
            nlmax = sm_pool.tile([128, 1], f32, tag="nlmax")
            nc.vector.tensor_reduce(
                out=nlmax[:w], in_=logits[:w], axis=mybir.AxisListType.X,
                op=AT.max, negate=True,
            )
            eb = sm_pool.tile([128, OUT_C], f32, tag="eb")
            esum = sm_pool.tile([128, 1], f32, tag="esum")
            nc.scalar.activation(
                eb[:w], logits[:w], Exp, bias=nlmax[:w], accum_out=esum[:w]
            )
            lse = sm_pool.tile([128, 1], f32, tag="lse")
            nc.scalar.activation(lse[:w], esum[:w], Ln)
            off = sm_pool.tile([128, 1], f32, tag="off")
            nc.vector.tensor_tensor(out=off[:w], in0=lse[:w], in1=nlmax[:w], op=AT.subtract)
            yy = sm_pool.tile([128, OUT_C], f32, tag="yy")
            nc.vector.tensor_tensor(
                out=yy[:w], in0=logits[:w],
                in1=off[:w].to_broadcast([w, OUT_C]), op=AT.subtract,
            )
            nc.sync.dma_start(y_d[j0 : j0 + w, :], yy[:w])

        _edge_loop(nc, tc, ctx, tps, ttp, ed_d, slot_t, iota_f, compute_msg, out_cb)
    _split_multi_waits(nc)
    return nc


# ---------------------------------------------------------------- host gather
def _u16(a):
    return a.view(np.uint16)


def _gather_ed(table_bf, qtab_bf, eidx, cidx, kv_w):
    """Build [128, TTP, kv_w + 64] bf16 per-edge array: kv rows from table
    (global ids), q row from qtab (core-local target ids)."""
    ttp = eidx.shape[1]
    out = np.empty((128, ttp, kv_w + HID), dtype=np.uint16)
    out[:, :, :kv_w] = _u16(table_bf)[eidx]
    out[:, :, kv_w:] = _u16(qtab_bf)[cidx]
    return out.view(BF16)


# ---------------------------------------------------------------- driver
def kernel(x, edge_index, lin1_w, lin1_b, wq, bq, wk, bk, wv, bv, lin2_w, lin2_b):
    _install_fixups()
    from concourse.bass_utils import run_bass_kernel_spmd

    x = np.asarray(x, dtype=np.float32)
    lin1_w = np.asarray(lin1_w, np.float32)
    lin1_b = np.asarray(lin1_b, np.float32)
    wq = np.asarray(wq, np.float32)
    bq = np.asarray(bq, np.float32)
    wk = np.asarray(wk, np.float32)
    bk = np.asarray(bk, np.float32)
    wv = np.asarray(wv, np.float32)
    bv = np.asarray(bv, np.float32)
    lin2_w = np.asarray(lin2_w, np.float32)
    lin2_b = np.asarray(lin2_b, np.float32)
    isd = np.float32(1.0 / np.sqrt(DH))

    metas, tps, ttp, s_all = _preprocess(np.asarray(edge_index))

    key = ("progs", tuple(tps.tolist()), ttp)
    if key not in _CACHE:
        _CACHE[key] = (
            _build_launch_A(),
            _build_launch_B(tps, ttp),
            _build_launch_C(tps, ttp),
            _build_launch_D(tps, ttp),
        )
    ncA, ncB, ncC, ncD = _CACHE[key]
    cores = list(range(NCORES))

    # ---- launch A: h = relu(x @ W1 + b1), columnar bf16
    xT = np.ascontiguousarray(x.T).astype(BF16)
    w1_bf = lin1_w.astype(BF16)
    in_maps = [
        dict(
            xT=np.ascontiguousarray(xT[:, c * NPC : (c + 1) * NPC]),
            w1=w1_bf,
            b1=lin1_b[:, None],
        )
        for c in cores
    ]
    resA = run_bass_kernel_spmd(ncA, in_maps, cores)
    hT = [np.asarray(resA.results[c]["hT_out"]) for c in cores]
    h_tab = np.ascontiguousarray(
        np.concatenate([t.T for t in hT], axis=0)
    )  # [N, 64] bf16

    # ---- launch B: layer 1 (attn == identity) + k2/v2/q2 tables
    s8 = [np.zeros((8, NPC), dtype=BF16) for _ in cores]
    for c in cores:
        s8[c][0] = s_all[c * NPC : (c + 1) * NPC].astype(BF16)
    bv08 = np.zeros((8, HID), dtype=BF16)
    bv08[0] = bv[0].astype(BF16)
    in_maps = [
        dict(
            ed=_u16(h_tab)[metas[c]["eidx"]].view(BF16),
            slot=metas[c]["slot"],
            nrmb=metas[c]["nrmb"],
            hT=hT[c],
            s8=s8[c],
            bv08=bv08,
            wv0=wv[0].astype(BF16),
            wk2=wk[1].astype(BF16),
            wv2=wv[1].astype(BF16),
            wq2=(wq[1] * isd).astype(BF16),
            bk2=bk[1][:, None],
            bv2=bv[1][:, None],
            bq2=(bq[1] * isd)[:, None],
        )
        for c in cores
    ]
    resB = run_bass_kernel_spmd(ncB, in_maps, cores)
    o1T = [np.asarray(resB.results[c]["outT"]) for c in cores]
    colsB = [np.asarray(resB.results[c]["cols"]) for c in cores]
    kv2_tab = np.ascontiguousarray(
        np.concatenate([cb[0:256].T for cb in colsB], axis=0)
    )  # [N, 256] = [k0 k1 v0 v1]
    q2_tab = [np.ascontiguousarray(cb[256:320].T) for cb in colsB]

    # ---- launch C: layer 2 + k3/v3/q3 tables
    in_maps = [
        dict(
            ed=_gather_ed(kv2_tab, q2_tab[c], metas[c]["eidx"], metas[c]["cidx"], 256),
            slot=metas[c]["slot"],
            nrm=metas[c]["nrm"],
            hT=hT[c],
            o1T=o1T[c],
            wk3=wk[2].astype(BF16),
            wv3=wv[2].astype(BF16),
            wq3=(wq[2] * isd).astype(BF16),
            bk3=bk[2][:, None],
            bv3=bv[2][:, None],
            bq3=(bq[2] * isd)[:, None],
        )
        for c in cores
    ]
    resC = run_bass_kernel_spmd(ncC, in_maps, cores)
    colsC = [np.asarray(resC.results[c]["cols"]) for c in cores]
    kv3_tab = np.ascontiguousarray(
        np.concatenate([cb[0:384].T for cb in colsC], axis=0)
    )  # [N, 384]
    q3_tab = [np.ascontiguousarray(cb[384:448].T) for cb in colsC]

    # ---- launch D: layer 3 + classifier head + log_softmax
    b2bc = np.ascontiguousarray(np.broadcast_to(lin2_b[None, :], (128, OUT_C)))
    in_maps = [
        dict(
            ed=_gather_ed(kv3_tab, q3_tab[c], metas[c]["eidx"], metas[c]["cidx"], 384),
            slot=metas[c]["slot"],
            nrm=metas[c]["nrm"],
            w2=lin2_w.astype(BF16),
            b2bc=b2bc,
        )
        for c in cores
    ]
    resD = run_bass_kernel_spmd(ncD, in_maps, cores)
    return np.concatenate(
        [np.asarray(resD.results[c]["y"], dtype=np.float32) for c in cores], axis=0
    )


# revision 10
# speedup vs baseline: 3.9026x; 1.1998x over previous
"""Trainium2 Bass kernel for 3-layer GNN message passing with per-edge
multi-head attention over node history, distributed over 8 NeuronCores.

Sharding: nodes are relabeled by descending degree and dealt into
(superblock, core, slot) so that each 128-edge tile maps partition p <->
target slot p ("identity segment" scheme): the segment-sum one-hot matrix
becomes a constant identity, q is per-superblock constant (no per-edge q
gather), and tiles per superblock = max in-degree within the superblock
(near-optimal padding). Per-edge k/v history rows are assembled on the host
between launches (pure indexing) and streamed as dense bf16; v tables are
d-major permuted (via host weight-column permutation) so the attention-apply
multiply runs in the DVE 2x mode. All FLOPs run on device. 4 launches:
proj, layer1, layer2, layer3+head.
"""

import sys
import types

import numpy as np
import ml_dtypes

sys.path.insert(0, "/opt/trn_rl_repo")

BF16 = ml_dtypes.bfloat16

# ---------------------------------------------------------------- fixups
_HOOK = [None]


def _install_fixups():
    if "antenv.axon_hooks" not in sys.modules:
        mod = types.ModuleType("antenv.axon_hooks")
        mod.set_axon_ntff_profile_hook = lambda h: _HOOK.__setitem__(0, h)
        mod.get_axon_ntff_profile_hook = lambda: _HOOK[0]
        sys.modules["antenv.axon_hooks"] = mod
        try:
            from trn_agent_boot.trn_boot import _ntff_profile_via_ctypes

            _HOOK[0] = _ntff_profile_via_ctypes("/opt/axon/libaxon_pjrt.so")
        except Exception:
            pass

    import concourse.tile as tile
    from concourse.vector_clock import ScopedClock
    import bass_rust

    if getattr(tile.TileContext, "_drain_split_installed", False):
        return

    def _drain_and_barrier(self, tick_clock, wait_clock):
        nc = self.nc
        drain_inst = nc.sync.drain()
        wait_clock.add_sem_waits(
            drain_inst.ins, ScopedClock({None: tick_clock.global_clock})
        )
        si = drain_inst.ins.sync_info
        waits = list(si.on_wait or []) if si is not None else []
        if len(waits) > 1:
            si.on_wait = waits[:1]
            for i in range(1, len(waits)):
                d2 = nc.sync.drain()
                d2.ins.sync_info = bass_rust.SyncInfo(
                    on_wait=waits[i : i + 1], on_update=[]
                )
        nc.all_engine_barrier()
        assert self.sems is not None
        popped = nc._tile_sem_poison_stack.pop()
        assert popped is self._sem_poison
        nc.clear_and_free_semaphores(list(self.sems.allocated().values()))
        nc.all_engine_barrier()

    tile.TileContext._drain_and_barrier = _drain_and_barrier
    tile.TileContext._drain_split_installed = True


# ---------------------------------------------------------------- constants
N = 20000
E = 320000
IN_C = 256
HID = 64
OUT_C = 64
HEADS = 8
DH = 8
NCORES = 8
NPC = N // NCORES  # 2500
SBT = 128  # target slots per superblock
NSB = (NPC + SBT - 1) // SBT  # 20 (last has 68 targets)
LASTW = NPC - (NSB - 1) * SBT  # 68
MAXG = 32  # max tiles per streamed chunk

# d-major permutation of the 64 features (8 heads x 8 dims), an involution
PRM = np.arange(HID).reshape(HEADS, DH).T.reshape(-1)

_CACHE = {}


# ---------------------------------------------------------------- host prep
def _preprocess(edge_index):
    row = np.asarray(edge_index[0], dtype=np.int64)
    col = np.asarray(edge_index[1], dtype=np.int64)
    loop = np.arange(N, dtype=np.int64)
    row_all = np.concatenate([row, loop])
    col_all = np.concatenate([col, loop])
    deg = np.bincount(col_all, minlength=N).astype(np.int64)
    dinv = (1.0 / np.sqrt(np.maximum(deg, 1))).astype(np.float32)
    norm = (dinv[row_all] * dinv[col_all]).astype(np.float32)
    s_all = np.bincount(col_all, weights=norm.astype(np.float64), minlength=N)
    s_all = s_all.astype(np.float32)

    # degree-sorted relabeling: rank r -> (superblock b, core c, slot p)
    order = np.argsort(-deg, kind="stable")  # global ids by desc degree
    b_of = np.empty(N, np.int64)
    c_of = np.empty(N, np.int64)
    p_of = np.empty(N, np.int64)
    ranks = np.arange(N)
    full = (NSB - 1) * 1024  # ranks dealt in blocks of 8*128
    b_of[ranks < full] = ranks[ranks < full] // 1024
    c_of[ranks < full] = (ranks[ranks < full] % 1024) // SBT
    p_of[ranks < full] = ranks[ranks < full] % SBT
    tail = ranks >= full
    b_of[tail] = NSB - 1
    c_of[tail] = (ranks[tail] - full) // LASTW
    p_of[tail] = (ranks[tail] - full) % LASTW
    # per-node placement (indexed by global id)
    nb = np.empty(N, np.int64); nb[order] = b_of
    ncr = np.empty(N, np.int64); ncr[order] = c_of
    npp = np.empty(N, np.int64); npp[order] = p_of
    # ids[c][b*128+p] = global id owned by core c at local index
    ids = np.empty((NCORES, NPC), np.int64)
    loc = nb * SBT + npp  # local index within core
    ids[ncr, loc] = np.arange(N)

    # tiles per superblock = max degree within the superblock (desc sorted)
    tps = np.zeros(NSB, np.int64)
    for b in range(NSB):
        r0 = b * 1024 if b < NSB - 1 else full
        tps[b] = max(1, int(deg[order[r0]]))
    sb_start = np.zeros(NSB + 1, np.int64)
    sb_start[1:] = np.cumsum(tps)
    tt = int(sb_start[-1])

    # scatter edges: edge i (sorted by target) lands at
    # core c(t), row p(t), column sb_start[b(t)] + within-target-rank
    es = np.argsort(col_all, kind="stable")
    tgt = col_all[es]
    src = row_all[es]
    nm = norm[es]
    start_of = np.zeros(N + 1, np.int64)
    start_of[1:] = np.cumsum(np.bincount(tgt, minlength=N))
    rank_in_tgt = np.arange(len(tgt)) - start_of[tgt]
    dcol = sb_start[nb[tgt]] + rank_in_tgt
    drow = npp[tgt]
    dcore = ncr[tgt]

    metas = []
    for c in range(NCORES):
        m = dcore == c
        eidx = np.zeros((128, tt), np.int64)
        nrm = np.zeros((128, tt), np.float32)
        eidx[drow[m], dcol[m]] = src[m]
        nrm[drow[m], dcol[m]] = nm[m]
        metas.append(dict(eidx=eidx, nrm=nrm, nrmb=nrm.astype(BF16)))

    # chunk plan: per sb, tiles split into chunks of <= MAXG; sbs processed
    # smallest-first so the pipeline ramps quickly
    chunks = []  # (sb, t0, gw, first, last)
    for b in np.argsort(tps, kind="stable"):
        b = int(b)
        t0 = int(sb_start[b])
        left = int(tps[b])
        while left > 0:
            gw = min(MAXG, left)
            chunks.append(
                (b, t0, gw, t0 == int(sb_start[b]), left == gw)
            )
            t0 += gw
            left -= gw
    return metas, tuple(int(x) for x in tps), tt, chunks, s_all, ids


_WS_CTR = [0]


def _split_multi_waits(nc, maxw=1):
    """This container's walrus rejects instructions with more than one sync
    wait; hoist excess waits onto NoOps inserted before the instruction."""
    from concourse import mybir

    for f in nc.m.functions:
        for bb in f.blocks:
            insts = list(bb.instructions)
            out = []
            changed = False
            for inst in insts:
                si = inst.sync_info
                waits = list(si.on_wait) if (si is not None and si.on_wait) else []
                if len(waits) > maxw:
                    excess = waits[: len(waits) - maxw]
                    for j in range(0, len(excess), maxw):
                        _WS_CTR[0] += 1
                        out.append(
                            mybir.InstNoOp(
                                name=f"waitsplit_{_WS_CTR[0]}",
                                engine=inst.engine,
                                sync_info=mybir.SyncInfo(
                                    on_wait=excess[j : j + maxw], on_update=[]
                                ),
                                bass_nofuse=True,
                            )
                        )
                    si.on_wait = waits[len(waits) - maxw :]
                    changed = True
                out.append(inst)
            if changed:
                bb.instructions = out


def _mk_nc():
    import concourse.bass as bass

    return bass.Bass(num_devices=NCORES, debug=False, target_bir_lowering=False)


def _load_w(nc, pool, dram_ap, p, f, tag, dtype=None):
    from concourse import mybir

    t = pool.tile([p, f], dtype or mybir.dt.float32, tag=tag)
    nc.sync.dma_start(t[:], dram_ap[:])
    return t


def _proj_cols(nc, tc, ctx, w_t, b_t, srcs, out_slices, act_pool, psum_pool):
    """For each (src columnar tile [64, NPC], dram slice): write
    (w.T @ src + b) in bf16 to the dram slice, chunked by 500 cols."""
    from concourse import mybir

    f32 = mybir.dt.float32
    bf = mybir.dt.bfloat16
    Ident = mybir.ActivationFunctionType.Identity
    NCH = 500
    for (src, dst) in zip(srcs, out_slices):
        for j0 in range(0, NPC, NCH):
            w = min(NCH, NPC - j0)
            ps = psum_pool.tile([HID, NCH], f32, tag="proj")
            nc.tensor.matmul(
                out=ps[:, :w], lhsT=w_t[:], rhs=src[:, j0 : j0 + w],
                start=True, stop=True,
            )
            sb = act_pool.tile([HID, NCH], bf, tag="projsb")
            nc.scalar.activation(sb[:, :w], ps[:, :w], Ident, bias=b_t[:])
            nc.sync.dma_start(dst[:, j0 : j0 + w], sb[:, :w])


def _proj_cols_f32(nc, tc, ctx, w_t, b_t, src, dst, act_pool, psum_pool):
    """Single projection written as f32 (for q tables that the next launch
    transposes on device)."""
    from concourse import mybir

    f32 = mybir.dt.float32
    Ident = mybir.ActivationFunctionType.Identity
    NCH = 500
    for j0 in range(0, NPC, NCH):
        w = min(NCH, NPC - j0)
        ps = psum_pool.tile([HID, NCH], f32, tag="projq")
        nc.tensor.matmul(
            out=ps[:, :w], lhsT=w_t[:], rhs=src[:, j0 : j0 + w],
            start=True, stop=True,
        )
        sb = act_pool.tile([HID, NCH], f32, tag="projqsb")
        nc.scalar.activation(sb[:, :w], ps[:, :w], Ident, bias=b_t[:])
        nc.sync.dma_start(dst[:, j0 : j0 + w], sb[:, :w])


def _consts(nc, tc, ctx):
    from concourse import mybir
    from concourse.masks import make_identity

    cpool = ctx.enter_context(tc.tile_pool(name="const", bufs=1))
    ident_f = cpool.tile([128, 128], mybir.dt.float32, tag="idf")
    make_identity(nc, ident_f[:])
    ident_b = cpool.tile([128, 128], mybir.dt.bfloat16, tag="idb")
    nc.vector.tensor_copy(ident_b[:], ident_f[:])
    return ident_f, ident_b


def _qrows_from_cols(nc, tc, ctx, qT_d, ident_f):
    """Load q column-table [64, NPC] f32, transpose per superblock into
    qrows_all [128, NSB, 64] bf16 (row p = q of slot p)."""
    from concourse import mybir

    f32 = mybir.dt.float32
    bf = mybir.dt.bfloat16
    qpool = ctx.enter_context(tc.tile_pool(name="q", bufs=1))
    qT = qpool.tile([HID, NPC], f32, tag="qT")
    nc.sync.dma_start(qT[:], qT_d[:])
    qrows = qpool.tile([128, NSB, HID], bf, tag="qrows")
    nc.vector.memset(qrows[:], 0.0)
    pst = ctx.enter_context(tc.tile_pool(name="pqt", bufs=2, space="PSUM"))
    for b in range(NSB):
        j0 = b * SBT
        w = min(SBT, NPC - j0)
        ps = pst.tile([128, HID], f32, tag="qtp")
        nc.tensor.transpose(
            out=ps[:w], in_=qT[:, j0 : j0 + w], identity=ident_f[:HID, :HID]
        )
        nc.scalar.copy(qrows[:w, b], ps[:w])
    return qrows


# ---------------------------------------------------------------- edge phase
def _edge_loop(nc, tc, ctx, chunks, ed_ap, ident_b, compute_msg, out_cb):
    """Stream per-sb chunks; compute_msg(ed_t, b, t0, gw) -> msg [128, gw, 64]
    bf16 (d-major features where applicable); segment-sum via identity matmul
    (psT[64, 128] = sum_tiles msg.T)."""
    from concourse import mybir

    f32 = mybir.dt.float32
    bf = mybir.dt.bfloat16
    roww = ed_ap.shape[2]
    ed_pool = ctx.enter_context(tc.tile_pool(name="ed", bufs=2))
    psum_seg = ctx.enter_context(tc.tile_pool(name="pseg", bufs=2, space="PSUM"))

    psT = None
    for (b, t0, gw, first, last) in chunks:
        ed_t = ed_pool.tile([128, MAXG, roww], bf, tag="ed")
        nc.sync.dma_start(ed_t[:, :gw], ed_ap[:, t0 : t0 + gw, :])
        msg = compute_msg(ed_t, b, t0, gw)
        if first:
            psT = psum_seg.tile([HID, 128], f32, tag="psT")
        for gi in range(gw):
            nc.tensor.matmul(
                out=psT[:],
                lhsT=msg[:, gi],
                rhs=ident_b[:],
                start=(first and gi == 0),
                stop=(last and gi == gw - 1),
            )
        if last:
            out_cb(b, psT)


def _nrm_tiles(nc, tc, ctx, tt, nrm_d=None, nrmb_d=None):
    from concourse import mybir

    meta_pool = ctx.enter_context(tc.tile_pool(name="meta", bufs=1))
    nrm_t = None
    if nrm_d is not None:
        nrm_t = meta_pool.tile([128, tt], mybir.dt.float32, tag="nrmf")
        nc.sync.dma_start(nrm_t[:], nrm_d[:])
    nrmb_t = None
    if nrmb_d is not None:
        nrmb_t = meta_pool.tile([128, tt], mybir.dt.bfloat16, tag="nrmb")
        nc.sync.dma_start(nrmb_t[:], nrmb_d[:])
    return nrm_t, nrmb_t


# ---------------------------------------------------------------- launch A
def _build_launch_A():
    import concourse.tile as tile
    from concourse import mybir
    from contextlib import ExitStack

    f32 = mybir.dt.float32
    bf = mybir.dt.bfloat16
    nc = _mk_nc()
    xT = nc.dram_tensor("xT", [IN_C, NPC], bf, kind="ExternalInput").ap()
    w1 = nc.dram_tensor("w1", [IN_C, HID], bf, kind="ExternalInput").ap()
    b1 = nc.dram_tensor("b1", [HID, 1], f32, kind="ExternalInput").ap()
    hT_out = nc.dram_tensor("hT_out", [HID, NPC], bf, kind="ExternalOutput").ap()

    with tile.TileContext(nc) as tc, ExitStack() as ctx:
        wpool = ctx.enter_context(tc.tile_pool(name="w", bufs=1))
        xpool = ctx.enter_context(tc.tile_pool(name="x", bufs=1))
        hpool = ctx.enter_context(tc.tile_pool(name="h", bufs=1))
        psum_pool = ctx.enter_context(tc.tile_pool(name="ps", bufs=2, space="PSUM"))

        w1a = _load_w(nc, wpool, w1[0:128, :], 128, HID, "w1a", bf)
        w1b = _load_w(nc, wpool, w1[128:256, :], 128, HID, "w1b", bf)
        b1t = _load_w(nc, wpool, b1, HID, 1, "b1t")
        xa = xpool.tile([128, NPC], bf, tag="xa")
        xb = xpool.tile([128, NPC], bf, tag="xb")
        nc.sync.dma_start(xa[:], xT[0:128, :])
        nc.sync.dma_start(xb[:], xT[128:256, :])

        hT = hpool.tile([HID, NPC], bf)
        NCH = 500
        Relu = mybir.ActivationFunctionType.Relu
        for j0 in range(0, NPC, NCH):
            w = min(NCH, NPC - j0)
            ps = psum_pool.tile([HID, NCH], f32, tag="p1")
            nc.tensor.matmul(out=ps[:, :w], lhsT=w1a[:], rhs=xa[:, j0 : j0 + w], start=True, stop=False)
            nc.tensor.matmul(out=ps[:, :w], lhsT=w1b[:], rhs=xb[:, j0 : j0 + w], start=False, stop=True)
            nc.scalar.activation(hT[:, j0 : j0 + w], ps[:, :w], Relu, bias=b1t[:])
        nc.sync.dma_start(hT_out[:], hT[:])
    _split_multi_waits(nc)
    return nc


# ---------------------------------------------------------------- launch B (layer 1)
def _build_launch_B(tt, chunks):
    import concourse.tile as tile
    from concourse import mybir
    from contextlib import ExitStack

    f32 = mybir.dt.float32
    bf = mybir.dt.bfloat16
    AT = mybir.AluOpType
    Relu = mybir.ActivationFunctionType.Relu
    nc = _mk_nc()

    ed_d = nc.dram_tensor("ed", [128, tt, HID], bf, kind="ExternalInput").ap()
    nrmb_d = nc.dram_tensor("nrmb", [128, tt], bf, kind="ExternalInput").ap()
    hT_d = nc.dram_tensor("hT", [HID, NPC], bf, kind="ExternalInput").ap()
    s8_d = nc.dram_tensor("s8", [8, NPC], bf, kind="ExternalInput").ap()
    bv08_d = nc.dram_tensor("bv08", [8, HID], bf, kind="ExternalInput").ap()
    wv0_d = nc.dram_tensor("wv0", [HID, HID], bf, kind="ExternalInput").ap()
    wk2_d = nc.dram_tensor("wk2", [HID, HID], bf, kind="ExternalInput").ap()
    wv2_d = nc.dram_tensor("wv2", [HID, HID], bf, kind="ExternalInput").ap()
    wq2_d = nc.dram_tensor("wq2", [HID, HID], bf, kind="ExternalInput").ap()
    bk2_d = nc.dram_tensor("bk2", [HID, 1], f32, kind="ExternalInput").ap()
    bv2_d = nc.dram_tensor("bv2", [HID, 1], f32, kind="ExternalInput").ap()
    bq2_d = nc.dram_tensor("bq2", [HID, 1], f32, kind="ExternalInput").ap()
    outT_d = nc.dram_tensor("outT", [HID, NPC], bf, kind="ExternalOutput").ap()
    cols_d = nc.dram_tensor("cols", [4 * HID, NPC], bf, kind="ExternalOutput").ap()
    q2T_d = nc.dram_tensor("q2T", [HID, NPC], f32, kind="ExternalOutput").ap()

    with tile.TileContext(nc) as tc, ExitStack() as ctx:
        ident_f, ident_b = _consts(nc, tc, ctx)
        _, nrmb_t = _nrm_tiles(nc, tc, ctx, tt, nrmb_d=nrmb_d)
        wpool = ctx.enter_context(tc.tile_pool(name="w", bufs=1))
        hpool = ctx.enter_context(tc.tile_pool(name="h", bufs=1))
        msg_pool = ctx.enter_context(tc.tile_pool(name="msg", bufs=2))
        act_pool = ctx.enter_context(tc.tile_pool(name="act", bufs=2))
        psum_o = ctx.enter_context(tc.tile_pool(name="po", bufs=2, space="PSUM"))
        psum_m = ctx.enter_context(tc.tile_pool(name="pm", bufs=2, space="PSUM"))

        wv0t = _load_w(nc, wpool, wv0_d, HID, HID, "wv0t", bf)
        wk2t = _load_w(nc, wpool, wk2_d, HID, HID, "wk2t", bf)
        wv2t = _load_w(nc, wpool, wv2_d, HID, HID, "wv2t", bf)
        wq2t = _load_w(nc, wpool, wq2_d, HID, HID, "wq2t", bf)
        bk2t = _load_w(nc, wpool, bk2_d, HID, 1, "bk2t")
        bv2t = _load_w(nc, wpool, bv2_d, HID, 1, "bv2t")
        bq2t = _load_w(nc, wpool, bq2_d, HID, 1, "bq2t")
        bv08t = _load_w(nc, wpool, bv08_d, 8, HID, "bv08t", bf)
        s8t = _load_w(nc, wpool, s8_d, 8, NPC, "s8t", bf)
        hT = hpool.tile([HID, NPC], bf, tag="hT")
        nc.sync.dma_start(hT[:], hT_d[:])
        outT = hpool.tile([HID, NPC], bf, tag="outT")

        def compute_msg(ed_t, b, t0, gw):
            msg = msg_pool.tile([128, MAXG, HID], bf, tag="msg")
            nc.vector.tensor_tensor(
                out=msg[:, :gw],
                in0=ed_t[:, :gw],
                in1=nrmb_t[:, t0 : t0 + gw, None].to_broadcast([128, gw, HID]),
                op=AT.mult,
            )
            return msg

        def out_cb(b, psT):
            j0 = b * SBT
            w = min(SBT, NPC - j0)
            ST = act_pool.tile([HID, 128], bf, tag="ST")
            nc.scalar.copy(ST[:, :w], psT[:, :w])
            ps2 = psum_o.tile([HID, 128], f32, tag="ps2")
            nc.tensor.matmul(out=ps2[:, :w], lhsT=wv0t[:], rhs=ST[:, :w], start=True, stop=False)
            nc.tensor.matmul(out=ps2[:, :w], lhsT=bv08t[:], rhs=s8t[:, j0 : j0 + w], start=False, stop=True)
            nc.scalar.activation(outT[:, j0 : j0 + w], ps2[:, :w], Relu)

        _edge_loop(nc, tc, ctx, chunks, ed_d, ident_b, compute_msg, out_cb)

        _proj_cols(
            nc, tc, ctx, wk2t, bk2t, [hT, outT],
            [cols_d[0:64, :], cols_d[64:128, :]], act_pool, psum_m,
        )
        _proj_cols(
            nc, tc, ctx, wv2t, bv2t, [hT, outT],
            [cols_d[128:192, :], cols_d[192:256, :]], act_pool, psum_m,
        )
        _proj_cols_f32(nc, tc, ctx, wq2t, bq2t, outT, q2T_d, act_pool, psum_m)
        nc.sync.dma_start(outT_d[:], outT[:])
    _split_multi_waits(nc)
    return nc


# ---------------------------------------------------------------- launch C (layer 2)
def _build_launch_C(tt, chunks):
    import concourse.tile as tile
    from concourse import mybir
    from contextlib import ExitStack

    f32 = mybir.dt.float32
    bf = mybir.dt.bfloat16
    AT = mybir.AluOpType
    Relu = mybir.ActivationFunctionType.Relu
    Sig = mybir.ActivationFunctionType.Sigmoid
    nc = _mk_nc()
    th = 2
    roww = 2 * th * HID  # 256: [k0 k1 | v0 v1(d-major)]

    ed_d = nc.dram_tensor("ed", [128, tt, roww], bf, kind="ExternalInput").ap()
    nrm_d = nc.dram_tensor("nrm", [128, tt], f32, kind="ExternalInput").ap()
    qT_d = nc.dram_tensor("qT", [HID, NPC], f32, kind="ExternalInput").ap()
    hT_d = nc.dram_tensor("hT", [HID, NPC], bf, kind="ExternalInput").ap()
    o1T_d = nc.dram_tensor("o1T", [HID, NPC], bf, kind="ExternalInput").ap()
    wk3a_d = nc.dram_tensor("wk3a", [HID, HID], bf, kind="ExternalInput").ap()
    wk3c_d = nc.dram_tensor("wk3c", [HID, HID], bf, kind="ExternalInput").ap()
    wv3a_d = nc.dram_tensor("wv3a", [HID, HID], bf, kind="ExternalInput").ap()
    wv3c_d = nc.dram_tensor("wv3c", [HID, HID], bf, kind="ExternalInput").ap()
    wq3_d = nc.dram_tensor("wq3", [HID, HID], bf, kind="ExternalInput").ap()
    bk3_d = nc.dram_tensor("bk3", [HID, 1], f32, kind="ExternalInput").ap()
    bv3_d = nc.dram_tensor("bv3", [HID, 1], f32, kind="ExternalInput").ap()
    bq3_d = nc.dram_tensor("bq3", [HID, 1], f32, kind="ExternalInput").ap()
    cols_d = nc.dram_tensor("cols", [6 * HID, NPC], bf, kind="ExternalOutput").ap()
    q3T_d = nc.dram_tensor("q3T", [HID, NPC], f32, kind="ExternalOutput").ap()

    with tile.TileContext(nc) as tc, ExitStack() as ctx:
        ident_f, ident_b = _consts(nc, tc, ctx)
        nrm_t, _ = _nrm_tiles(nc, tc, ctx, tt, nrm_d=nrm_d)
        qrows = _qrows_from_cols(nc, tc, ctx, qT_d, ident_f)
        wpool = ctx.enter_context(tc.tile_pool(name="w", bufs=1))
        hpool = ctx.enter_context(tc.tile_pool(name="h", bufs=1))
        dk_pool = ctx.enter_context(tc.tile_pool(name="dk", bufs=2))
        sc_pool = ctx.enter_context(tc.tile_pool(name="sc", bufs=2))
        msg_pool = ctx.enter_context(tc.tile_pool(name="msg", bufs=2))
        act_pool = ctx.enter_context(tc.tile_pool(name="act", bufs=2))
        psum_m = ctx.enter_context(tc.tile_pool(name="pm", bufs=2, space="PSUM"))

        wk3at = _load_w(nc, wpool, wk3a_d, HID, HID, "wk3at", bf)
        wk3ct = _load_w(nc, wpool, wk3c_d, HID, HID, "wk3ct", bf)
        wv3at = _load_w(nc, wpool, wv3a_d, HID, HID, "wv3at", bf)
        wv3ct = _load_w(nc, wpool, wv3c_d, HID, HID, "wv3ct", bf)
        wq3t = _load_w(nc, wpool, wq3_d, HID, HID, "wq3t", bf)
        bk3t = _load_w(nc, wpool, bk3_d, HID, 1, "bk3t")
        bv3t = _load_w(nc, wpool, bv3_d, HID, 1, "bv3t")
        bq3t = _load_w(nc, wpool, bq3_d, HID, 1, "bq3t")
        hT = hpool.tile([HID, NPC], bf, tag="hT")
        nc.sync.dma_start(hT[:], hT_d[:])
        o1T = hpool.tile([HID, NPC], bf, tag="o1T")
        nc.sync.dma_start(o1T[:], o1T_d[:])
        o2T = hpool.tile([HID, NPC], bf, tag="o2T")

        def compute_msg(ed_t, b, t0, gw):
            ke = ed_t[:, :gw, 0 : 2 * HID].rearrange("p c (t d) -> p c t d", t=2)
            dk = dk_pool.tile([128, MAXG, 2, HID], bf, tag="dk")
            nc.vector.tensor_tensor(
                out=dk[:, :gw],
                in0=ke,
                in1=qrows[:, b : b + 1, None, :].to_broadcast([128, gw, 2, HID]),
                op=AT.mult,
            )
            sc = sc_pool.tile([128, MAXG, 2, 8], f32, tag="sc")
            nc.vector.tensor_reduce(
                out=sc[:, :gw],
                in_=dk[:, :gw].rearrange("p c t (h d) -> p c t h d", h=8),
                axis=mybir.AxisListType.X,
                op=AT.add,
            )
            z = sc_pool.tile([128, MAXG, 8], f32, tag="z")
            nc.vector.tensor_tensor(out=z[:, :gw], in0=sc[:, :gw, 0], in1=sc[:, :gw, 1], op=AT.subtract)
            a0 = sc_pool.tile([128, MAXG, 8], f32, tag="a0")
            nc.scalar.activation(a0[:, :gw], z[:, :gw], Sig)
            a1 = sc_pool.tile([128, MAXG, 8], f32, tag="a1")
            nc.scalar.activation(a1[:, :gw], z[:, :gw], Sig, scale=-1.0)
            an0 = sc_pool.tile([128, MAXG, 8], bf, tag="an0")
            nc.vector.tensor_tensor(
                out=an0[:, :gw], in0=a0[:, :gw],
                in1=nrm_t[:, t0 : t0 + gw, None].to_broadcast([128, gw, 8]),
                op=AT.mult,
            )
            an1 = sc_pool.tile([128, MAXG, 8], bf, tag="an1")
            nc.vector.tensor_tensor(
                out=an1[:, :gw], in0=a1[:, :gw],
                in1=nrm_t[:, t0 : t0 + gw, None].to_broadcast([128, gw, 8]),
                op=AT.mult,
            )
            # v is d-major: [128, c, t, 8d, 8h]; an broadcast on the d axis
            ve = ed_t[:, :gw, 2 * HID : 4 * HID].rearrange(
                "p c (t d h) -> p c t d h", t=2, d=8
            )
            wv0_ = msg_pool.tile([128, MAXG, 8, 8], bf, tag="wv0")
            nc.vector.tensor_tensor(
                out=wv0_[:, :gw], in0=ve[:, :, 0],
                in1=an0[:, :gw, None, :].to_broadcast([128, gw, 8, 8]),
                op=AT.mult,
            )
            wv1_ = msg_pool.tile([128, MAXG, 8, 8], bf, tag="wv1")
            nc.vector.tensor_tensor(
                out=wv1_[:, :gw], in0=ve[:, :, 1],
                in1=an1[:, :gw, None, :].to_broadcast([128, gw, 8, 8]),
                op=AT.mult,
            )
            msg = msg_pool.tile([128, MAXG, HID], bf, tag="msg")
            nc.vector.tensor_tensor(
                out=msg[:, :gw].rearrange("p c (d h) -> p c d h", d=8),
                in0=wv0_[:, :gw], in1=wv1_[:, :gw], op=AT.add,
            )
            return msg

        def out_cb(b, psT):
            j0 = b * SBT
            w = min(SBT, NPC - j0)
            nc.scalar.activation(o2T[:, j0 : j0 + w], psT[:, :w], Relu)

        _edge_loop(nc, tc, ctx, chunks, ed_d, ident_b, compute_msg, out_cb)

        # k tables: hT/o1T natural-in; o2T d-major-in (wk3c pre-permuted)
        _proj_cols(
            nc, tc, ctx, wk3at, bk3t, [hT, o1T],
            [cols_d[0:64, :], cols_d[64:128, :]], act_pool, psum_m,
        )
        _proj_cols(
            nc, tc, ctx, wk3ct, bk3t, [o2T], [cols_d[128:192, :]], act_pool, psum_m,
        )
        # v tables: d-major-out (weights col-permuted); wv3c also row-permuted
        _proj_cols(
            nc, tc, ctx, wv3at, bv3t, [hT, o1T],
            [cols_d[192:256, :], cols_d[256:320, :]], act_pool, psum_m,
        )
        _proj_cols(
            nc, tc, ctx, wv3ct, bv3t, [o2T], [cols_d[320:384, :]], act_pool, psum_m,
        )
        _proj_cols_f32(nc, tc, ctx, wq3t, bq3t, o2T, q3T_d, act_pool, psum_m)
    _split_multi_waits(nc)
    return nc


# ---------------------------------------------------------------- launch D (layer 3 + head)
def _build_launch_D(tt, chunks):
    import concourse.tile as tile
    from concourse import mybir
    from contextlib import ExitStack

    f32 = mybir.dt.float32
    bf = mybir.dt.bfloat16
    AT = mybir.AluOpType
    Relu = mybir.ActivationFunctionType.Relu
    Exp = mybir.ActivationFunctionType.Exp
    Ln = mybir.ActivationFunctionType.Ln
    nc = _mk_nc()
    th = 3
    roww = 2 * th * HID  # 384: [k0 k1 k2 | v0 v1 v2(d-major)]

    ed_d = nc.dram_tensor("ed", [128, tt, roww], bf, kind="ExternalInput").ap()
    nrm_d = nc.dram_tensor("nrm", [128, tt], f32, kind="ExternalInput").ap()
    qT_d = nc.dram_tensor("qT", [HID, NPC], f32, kind="ExternalInput").ap()
    w2_d = nc.dram_tensor("w2", [HID, OUT_C], bf, kind="ExternalInput").ap()
    b2bc_d = nc.dram_tensor("b2bc", [128, OUT_C], f32, kind="ExternalInput").ap()
    y_d = nc.dram_tensor("y", [NPC, OUT_C], f32, kind="ExternalOutput").ap()

    with tile.TileContext(nc) as tc, ExitStack() as ctx:
        ident_f, ident_b = _consts(nc, tc, ctx)
        nrm_t, _ = _nrm_tiles(nc, tc, ctx, tt, nrm_d=nrm_d)
        qrows = _qrows_from_cols(nc, tc, ctx, qT_d, ident_f)
        wpool = ctx.enter_context(tc.tile_pool(name="w", bufs=1))
        dk_pool = ctx.enter_context(tc.tile_pool(name="dk", bufs=2))
        sc_pool = ctx.enter_context(tc.tile_pool(name="sc", bufs=2))
        msg_pool = ctx.enter_context(tc.tile_pool(name="msg", bufs=2))
        act_pool = ctx.enter_context(tc.tile_pool(name="act", bufs=2))
        sm_pool = ctx.enter_context(tc.tile_pool(name="sm", bufs=2))
        psum_lg = ctx.enter_context(tc.tile_pool(name="plg", bufs=2, space="PSUM"))

        w2t = _load_w(nc, wpool, w2_d, HID, OUT_C, "w2t", bf)
        b2t = _load_w(nc, wpool, b2bc_d, 128, OUT_C, "b2t")

        def compute_msg(ed_t, b, t0, gw):
            ke = ed_t[:, :gw, 0 : 3 * HID].rearrange("p c (t d) -> p c t d", t=3)
            dk = dk_pool.tile([128, MAXG, 3, HID], bf, tag="dk")
            nc.vector.tensor_tensor(
                out=dk[:, :gw],
                in0=ke,
                in1=qrows[:, b : b + 1, None, :].to_broadcast([128, gw, 3, HID]),
                op=AT.mult,
            )
            sc = sc_pool.tile([128, MAXG, 3, 8], f32, tag="sc")
            nc.vector.tensor_reduce(
                out=sc[:, :gw],
                in_=dk[:, :gw].rearrange("p c t (h d) -> p c t h d", h=8),
                axis=mybir.AxisListType.X,
                op=AT.add,
            )
            ee = sc_pool.tile([128, MAXG, 3, 8], bf, tag="ee")
            nc.scalar.activation(ee[:, :gw], sc[:, :gw], Exp)
            dd1 = sc_pool.tile([128, MAXG, 8], bf, tag="dd1")
            nc.vector.tensor_tensor(out=dd1[:, :gw], in0=ee[:, :gw, 0], in1=ee[:, :gw, 1], op=AT.add)
            dd = sc_pool.tile([128, MAXG, 8], f32, tag="dd")
            nc.vector.tensor_tensor(out=dd[:, :gw], in0=dd1[:, :gw], in1=ee[:, :gw, 2], op=AT.add)
            rr = sc_pool.tile([128, MAXG, 8], f32, tag="rr")
            nc.vector.reciprocal(rr[:, :gw], dd[:, :gw])
            rn = sc_pool.tile([128, MAXG, 8], bf, tag="rn")
            nc.vector.tensor_tensor(
                out=rn[:, :gw], in0=rr[:, :gw],
                in1=nrm_t[:, t0 : t0 + gw, None].to_broadcast([128, gw, 8]),
                op=AT.mult,
            )
            aa = sc_pool.tile([128, MAXG, 3, 8], bf, tag="aa")
            nc.vector.tensor_tensor(
                out=aa[:, :gw], in0=ee[:, :gw],
                in1=rn[:, :gw, None, :].to_broadcast([128, gw, 3, 8]),
                op=AT.mult,
            )
            ve = ed_t[:, :gw, 3 * HID : 6 * HID].rearrange(
                "p c (t d h) -> p c t d h", t=3, d=8
            )
            wv_ = msg_pool.tile([128, MAXG, 3, 8, 8], bf, tag="wv")
            nc.vector.tensor_tensor(
                out=wv_[:, :gw], in0=ve,
                in1=aa[:, :gw, :, None, :].to_broadcast([128, gw, 3, 8, 8]),
                op=AT.mult,
            )
            msg1 = msg_pool.tile([128, MAXG, 8, 8], bf, tag="msg1")
            nc.vector.tensor_tensor(out=msg1[:, :gw], in0=wv_[:, :gw, 0], in1=wv_[:, :gw, 1], op=AT.add)
            msg = msg_pool.tile([128, MAXG, HID], bf, tag="msg")
            nc.vector.tensor_tensor(
                out=msg[:, :gw].rearrange("p c (d h) -> p c d h", d=8),
                in0=msg1[:, :gw], in1=wv_[:, :gw, 2], op=AT.add,
            )
            return msg

        def out_cb(b, psT):
            j0 = b * SBT
            w = min(SBT, NPC - j0)
            o3T = act_pool.tile([HID, 128], bf, tag="o3T")
            nc.scalar.activation(o3T[:, :w], psT[:, :w], Relu)
            lg = psum_lg.tile([128, OUT_C], f32, tag="lg")
            nc.tensor.matmul(out=lg[:w], lhsT=o3T[:, :w], rhs=w2t[:], start=True, stop=True)
            logits = sm_pool.tile([128, OUT_C], f32, tag="logits")
            nc.vector.tensor_tensor(out=logits[:w], in0=lg[:w], in1=b2t[:w], op=AT.add)
            nlmax = sm_pool.tile([128, 1], f32, tag="nlmax")
            nc.vector.tensor_reduce(
                out=nlmax[:w], in_=logits[:w], axis=mybir.AxisListType.X,
                op=AT.max, negate=True,
            )
            eb = sm_pool.tile([128, OUT_C], f32, tag="eb")
            esum = sm_pool.tile([128, 1], f32, tag="esum")
            nc.scalar.activation(
                eb[:w], logits[:w], Exp, bias=nlmax[:w], accum_out=esum[:w]
            )
            lse = sm_pool.tile([128, 1], f32, tag="lse")
            nc.scalar.activation(lse[:w], esum[:w], Ln)
            off = sm_pool.tile([128, 1], f32, tag="off")
            nc.vector.tensor_tensor(out=off[:w], in0=lse[:w], in1=nlmax[:w], op=AT.subtract)
            yy = sm_pool.tile([128, OUT_C], f32, tag="yy")
            nc.vector.tensor_tensor(
                out=yy[:w], in0=logits[:w],
                in1=off[:w].to_broadcast([w, OUT_C]), op=AT.subtract,
            )
            nc.sync.dma_start(y_d[j0 : j0 + w, :], yy[:w])

        _edge_loop(nc, tc, ctx, chunks, ed_d, ident_b, compute_msg, out_cb)
    _split_multi_waits(nc)
    return nc


# ---------------------------------------------------------------- host gather
def _u16(a):
    return a.view(np.uint16)


def _gather_ed(ktab, vtab, eidx):
    """[128, TT, 2*th*64] bf16: k rows then v rows (both by global src id)."""
    tt = eidx.shape[1]
    kw = ktab.shape[1]
    vw = vtab.shape[1]
    out = np.empty((128, tt, kw + vw), dtype=np.uint16)
    out[:, :, :kw] = _u16(ktab)[eidx]
    out[:, :, kw:] = _u16(vtab)[eidx]
    return out.view(BF16)


def _scatter_tab(cols_list, ids, lo, hi, dtype=BF16):
    """tab[global_id] = cols[lo:hi].T for each core."""
    tab = np.empty((N, hi - lo), dtype=dtype)
    for c in range(NCORES):
        tab[ids[c]] = cols_list[c][lo:hi].T
    return tab


# ---------------------------------------------------------------- driver
def kernel(x, edge_index, lin1_w, lin1_b, wq, bq, wk, bk, wv, bv, lin2_w, lin2_b):
    _install_fixups()
    from concourse.bass_utils import run_bass_kernel_spmd

    x = np.asarray(x, dtype=np.float32)
    lin1_w = np.asarray(lin1_w, np.float32)
    lin1_b = np.asarray(lin1_b, np.float32)
    wq = np.asarray(wq, np.float32)
    bq = np.asarray(bq, np.float32)
    wk = np.asarray(wk, np.float32)
    bk = np.asarray(bk, np.float32)
    wv = np.asarray(wv, np.float32)
    bv = np.asarray(bv, np.float32)
    lin2_w = np.asarray(lin2_w, np.float32)
    lin2_b = np.asarray(lin2_b, np.float32)
    isd = np.float32(1.0 / np.sqrt(DH))

    metas, tps, tt, chunks, s_all, ids = _preprocess(np.asarray(edge_index))

    key = ("progs", tps, tt)
    if key not in _CACHE:
        _CACHE[key] = (
            _build_launch_A(),
            _build_launch_B(tt, chunks),
            _build_launch_C(tt, chunks),
            _build_launch_D(tt, chunks),
        )
    ncA, ncB, ncC, ncD = _CACHE[key]
    cores = list(range(NCORES))

    # ---- launch A: h = relu(x @ W1 + b1), columnar bf16
    xT = np.ascontiguousarray(x.T).astype(BF16)
    w1_bf = lin1_w.astype(BF16)
    in_maps = [
        dict(
            xT=np.ascontiguousarray(xT[:, ids[c]]),
            w1=w1_bf,
            b1=lin1_b[:, None],
        )
        for c in cores
    ]
    resA = run_bass_kernel_spmd(ncA, in_maps, cores)
    hT = [np.asarray(resA.results[c]["hT_out"]) for c in cores]
    h_tab = np.empty((N, HID), dtype=BF16)
    for c in cores:
        h_tab[ids[c]] = hT[c].T

    # ---- launch B: layer 1 (attn == identity) + k2/v2/q2 tables
    s8 = []
    for c in cores:
        a = np.zeros((8, NPC), dtype=BF16)
        a[0] = s_all[ids[c]].astype(BF16)
        s8.append(a)
    bv08 = np.zeros((8, HID), dtype=BF16)
    bv08[0] = bv[0].astype(BF16)
    in_maps = [
        dict(
            ed=_u16(h_tab)[metas[c]["eidx"]].view(BF16),
            nrmb=metas[c]["nrmb"],
            hT=hT[c],
            s8=s8[c],
            bv08=bv08,
            wv0=wv[0].astype(BF16),
            wk2=wk[1].astype(BF16),
            wv2=wv[1][:, PRM].astype(BF16),
            wq2=(wq[1] * isd).astype(BF16),
            bk2=bk[1][:, None],
            bv2=bv[1][PRM][:, None],
            bq2=(bq[1] * isd)[:, None],
        )
        for c in cores
    ]
    resB = run_bass_kernel_spmd(ncB, in_maps, cores)
    o1T = [np.asarray(resB.results[c]["outT"]) for c in cores]
    colsB = [np.asarray(resB.results[c]["cols"]) for c in cores]
    q2T = [np.asarray(resB.results[c]["q2T"]) for c in cores]
    ktab2 = _scatter_tab(colsB, ids, 0, 128)
    vtab2 = _scatter_tab(colsB, ids, 128, 256)

    # ---- launch C: layer 2 + k3/v3/q3 tables
    in_maps = [
        dict(
            ed=_gather_ed(ktab2, vtab2, metas[c]["eidx"]),
            nrm=metas[c]["nrm"],
            qT=q2T[c],
            hT=hT[c],
            o1T=o1T[c],
            wk3a=wk[2].astype(BF16),
            wk3c=wk[2][PRM, :].astype(BF16),
            wv3a=wv[2][:, PRM].astype(BF16),
            wv3c=wv[2][PRM, :][:, PRM].astype(BF16),
            wq3=((wq[2] * isd)[PRM, :]).astype(BF16),
            bk3=bk[2][:, None],
            bv3=bv[2][PRM][:, None],
            bq3=(bq[2] * isd)[:, None],
        )
        for c in cores
    ]
    resC = run_bass_kernel_spmd(ncC, in_maps, cores)
    colsC = [np.asarray(resC.results[c]["cols"]) for c in cores]
    q3T = [np.asarray(resC.results[c]["q3T"]) for c in cores]
    ktab3 = _scatter_tab(colsC, ids, 0, 192)
    vtab3 = _scatter_tab(colsC, ids, 192, 384)

    # ---- launch D: layer 3 + classifier head + log_softmax
    b2bc = np.ascontiguousarray(np.broadcast_to(lin2_b[None, :], (128, OUT_C)))
    in_maps = [
        dict(
            ed=_gather_ed(ktab3, vtab3, metas[c]["eidx"]),
            nrm=metas[c]["nrm"],
            qT=q3T[c],
            w2=lin2_w[PRM, :].astype(BF16),
            b2bc=b2bc,
        )
        for c in cores
    ]
    resD = run_bass_kernel_spmd(ncD, in_maps, cores)
    y = np.empty((N, OUT_C), dtype=np.float32)
    for c in cores:
        y[ids[c]] = np.asarray(resD.results[c]["y"], dtype=np.float32)
    return y
